# revision 23
# baseline (speedup 1.0000x reference)
"""Trainium2 Bass kernel for nn_GCNN_87668872446200.

Branch-split design over 8 NeuronCores: cores 0-3 run protein branch 1,
cores 4-7 run branch 2.  Within a branch group each core owns a quarter of
the destination nodes and the full F=1024 feature dim.

Per core (fp8 e4m3 data paths, DoubleRow fp8 matmuls):
  - xw' = 8*(x*dinv_row) @ (W*64) / 8   computed on PE in two source-halves,
    written to HBM as two tensors (xwA rows <5120 + bias row, xwB rest)
  - symmetric norm is separated: h = Dinv A Dinv xw + b realized as
    S-matmul with S[e,d] = dinv[d] (bias via a virtual edge to a bias row)
  - dma_gather pulls 1KB fp8 rows per edge; source-half split lets the
    Q7 descriptor emission of half A overlap the xw compute of half B
  - phase A partial sums staged in SBUF (fp8), injected into phase B PSUM
    via an identity matmul; one ACT pass does lrelu(psum/8)
  - per-graph mean-pool as PE matmul (mpool*256 fp8), W_pf applied locally
  - masif branch (8 graphs/core, this core's branch only)
  - one small AllReduce ([3,128,32] f32 = 48KB) + replicated dense head

All 8 cores run ONE identical program; per-core variation is in input data.
"""
import numpy as np

# ---------------------------------------------------------------- constants
N_CORES = 8
P = 128
BLK = 128           # dest nodes per block
NQ = 4              # dest quarters per branch group
GRPU = 8            # 128-idx units per gather call (1024 idxs)

N_NODES, N_EDGES, F_DIM, B_GRAPHS, L_MAS, C_MAS = 10000, 80000, 1024, 32, 800, 16

USE_DR = True       # DoubleRow fp8 matmuls


class _Cfg:
    def __init__(self, n=N_NODES, e=N_EDGES, f=F_DIM, b=B_GRAPHS,
                 l=L_MAS, c=C_MAS):
        self.N, self.E, self.F, self.B, self.L, self.C = n, e, f, b, l, c
        self.NPAD = ((n + 511) // 512) * 512          # 10240
        self.QH = self.NPAD // NQ                     # 2560 dests per core
        self.NBLK = self.QH // BLK                    # 20 blocks
        self.SH = self.NPAD // 2                      # 5120 source-half split
        self.KC2 = f // 256                           # 4 k-pairs
        self.GPB = b // 4                             # 8 graphs per core
        self.LW = l // 80                             # 10
        self.LB = 8                                   # l-blocks
        self.LBS = l // self.LB                       # 100
        self.WPB = self.LBS // self.LW                # 10
        # xwA holds source rows 0..SH-1 plus bias row (SH) and zero row (SH+1)
        self.XWA_ROWS = self.SH + P                   # 5248
        self.XWB_ROWS = self.NPAD - self.SH           # 5120 (tail rows zero)


def _q8(x):
    import ml_dtypes
    return np.clip(np.asarray(x, np.float32), -240.0, 240.0).astype(
        ml_dtypes.float8_e4m3)


# ---------------------------------------------------------------- host prep
def _edge_plan_core(cfg, edge_index, q):
    """Edges targeting quarter q, split per (block, source-half), sorted.
    Returns dict (j, hf) -> (rows, dests, counts)."""
    row = np.asarray(edge_index[0]).astype(np.int64)
    col = np.asarray(edge_index[1]).astype(np.int64)
    loops = np.arange(cfg.N, dtype=np.int64)
    rows = np.concatenate([row, loops])
    cols = np.concatenate([col, loops])
    lo, hi = q * cfg.QH, (q + 1) * cfg.QH
    sel = (cols >= lo) & (cols < hi)
    r, c = rows[sel], cols[sel] - lo
    out = {}
    for j in range(cfg.NBLK):
        bsel = (c >= j * BLK) & (c < (j + 1) * BLK)
        rj, cj = r[bsel], c[bsel] - j * BLK
        for hf in range(2):
            hsel = (rj < cfg.SH) if hf == 0 else (rj >= cfg.SH)
            out[(j, hf)] = (rj[hsel], cj[hsel])
    return out


def _shared_schedule(cfg, plans):
    """Shared chunk schedule (max over the 8 per-core plans).

    Returns chunks: list of dicts with keys
      hf, j, units (1 or 2), u0 (unit offset in group), grp (group index),
      first (starts block), last (ends block's half... block completion is
      tracked at (j,hf==1,last) for B and (j,hf==0,last) for A)
    and n_groups_a / n_groups_b.
    """
    # per-plan unit needs and slot permutation (sort blocks big->small so the
    # max-over-cores slot schedule aligns; mpool/idx/smat are per-core data)
    def units_of(p, j, hf):
        e = len(p[(j, hf)][0]) + (1 if hf == 0 else 0)
        return max(1, (e + P - 1) // P)

    perms = []
    for p in plans:
        tot = [units_of(p, j, 0) + units_of(p, j, 1) for j in range(cfg.NBLK)]
        perms.append(list(np.argsort(-np.asarray(tot), kind='stable')))

    slot_need = {}
    for s in range(cfg.NBLK):
        for hf in range(2):
            slot_need[(s, hf)] = max(
                units_of(p, perm[s], hf) for p, perm in zip(plans, perms))

    chunks = []
    groups = []                                   # list of [hf, nunits]
    for hf in range(2):
        space = 0                                 # force new group per half
        for s in range(cfg.NBLK):
            left = slot_need[(s, hf)]
            first = True
            while left:
                if space == 0:
                    groups.append([hf, 0])
                    space = GRPU
                sz = 2 if (left >= 2 and space >= 2) else 1
                chunks.append(dict(hf=hf, j=s, units=sz, u0=GRPU - space,
                                   grp=len(groups) - 1,
                                   first=first, last=(left - sz == 0)))
                space -= sz
                left -= sz
                first = False
                groups[-1][1] = GRPU - space
    return chunks, groups, perms


def _fill_core_gather2(cfg, chunks, groups, plan, dinv, q, perm):
    """Per-core idx + smat content for the shared schedule (slot j maps to
    physical block perm[j] for this core)."""
    n_groups = len(groups)
    flat_idx = np.zeros((n_groups, GRPU * P), np.int64)
    smat = np.zeros((n_groups, P, GRPU * P), np.float32)
    dinv8 = _q8(dinv).astype(np.float32)
    consumed = {}
    for ch in chunks:
        sl, hf, g, u0 = ch['j'], ch['hf'], ch['grp'], ch['u0']
        j = perm[sl]
        r, c = plan[(j, hf)]
        off = consumed.get((j, hf), 0)
        cap = ch['units'] * P
        base = u0 * P
        pad_idx = cfg.SH + 1 if hf == 0 else cfg.XWB_ROWS - 1
        flat_idx[g, base:base + cap] = pad_idx
        s = 0
        if hf == 0 and ch['first']:
            flat_idx[g, base] = cfg.SH            # bias row at slot 0
            smat[g, 0, base:base + P] = 1.0
            s = 1
        take = min(len(r) - off, cap - s)
        if take > 0:
            rr = r[off:off + take]
            cc = c[off:off + take]
            if hf == 1:
                rr = rr - cfg.SH
            slots = np.arange(s, s + take) + base
            up = slots // P
            pp = slots % P
            flat_idx[g, slots] = rr
            # dest scale dinv[global dest] ; global dest = q*QH + j*BLK + cc
            gd = q * cfg.QH + j * BLK + cc
            vals = dinv8[np.minimum(gd, cfg.N - 1)] * (gd < cfg.N)
            smat[g, pp, up * P + cc] = vals
        consumed[(j, hf)] = off + take
    for (j, hf), off in consumed.items():
        assert off == len(plan[(j, hf)][0]), (j, hf, off, len(plan[(j, hf)][0]))
    return flat_idx, smat


def _wrap_idx_groups(flat_idx):
    """[G, 1024] -> [128, G*64] int16 (16-part wrap, 8x replicated)."""
    g, n = flat_idx.shape
    w = flat_idx.reshape(g, n // 16, 16).transpose(2, 0, 1).reshape(16, -1)
    return np.tile(w, (8, 1)).astype(np.int16)


def _preprocess(inputs, cfg):
    import ml_dtypes
    bf16 = ml_dtypes.bfloat16
    f32 = np.float32

    # --- per-branch shared data
    bdata = {}
    for br in (1, 2):
        x = np.asarray(inputs[f'pro{br}_x'], f32)
        ei = np.asarray(inputs[f'pro{br}_edge_index'])
        batch = np.asarray(inputs[f'pro{br}_batch']).astype(np.int64)
        row = ei[0].astype(np.int64)
        col = ei[1].astype(np.int64)
        deg = np.bincount(np.concatenate([col, np.arange(cfg.N)]),
                          minlength=cfg.N).astype(np.float64)
        dinv = (1.0 / np.sqrt(deg)).astype(f32)
        # xt_dr [NSLAB, 128, KC2*2*512] fp8 of (x*dinv_row)^T, slab-major
        xp = x * dinv[:, None]
        xpT = np.zeros((cfg.F, cfg.NPAD), f32)
        xpT[:, :cfg.N] = xp.T
        nslab = cfg.NPAD // 512
        xt_dr = np.ascontiguousarray(
            xpT.reshape(cfg.KC2, 2, P, nslab, 512).transpose(3, 2, 0, 1, 4)
        ).reshape(nslab, P, cfg.KC2 * 2 * 512)
        # wg_dr [128, KC2*2*F] fp8  (p, c, i, f)
        W = np.asarray(inputs[f'W_g{br}'], f32) * 64.0
        wg_dr = np.ascontiguousarray(
            W.reshape(cfg.KC2, 2, P, cfg.F).transpose(2, 0, 1, 3)
        ).reshape(P, cfg.KC2 * 2 * cfg.F)
        b8 = np.asarray(inputs[f'b_g{br}'], f32) * 8.0
        cnt = np.bincount(batch, minlength=cfg.B).astype(f32)
        plans = [_edge_plan_core(cfg, ei, q) for q in range(NQ)]
        bdata[br] = dict(xt=_q8(xt_dr), wg=_q8(wg_dr), b8=_q8(b8[None, :]),
                         dinv=dinv, batch=batch, cnt=cnt, plans=plans)

    # --- shared chunk schedule (max over all 8 core plans, slot-permuted)
    all_plans = bdata[1]['plans'] + bdata[2]['plans']
    chunks, groups, perms = _shared_schedule(cfg, all_plans)
    n_groups = len(groups)
    n_ga = sum(1 for g in groups if g[0] == 0)

    meta = dict(chunks=chunks, groups=groups, n_groups=n_groups, n_ga=n_ga)

    # --- head weights (shared across cores)
    shared = {}
    shared['wfc1'] = np.ascontiguousarray(
        np.asarray(inputs['W_fc1'], f32).reshape(2, P, 256).transpose(1, 0, 2))
    shared['wfc2'] = np.ascontiguousarray(
        np.asarray(inputs['W_fc2'], f32).reshape(2, P, 64).transpose(1, 0, 2))
    shared['bfc1'] = np.ascontiguousarray(
        np.asarray(inputs['b_fc1'], f32).reshape(2, P, 1).transpose(1, 0, 2))
    shared['bfc2'] = np.asarray(inputs['b_fc2'], f32).reshape(64, 1)
    wout = np.asarray(inputs['W_out'], f32)
    shared['wout_x'] = np.ascontiguousarray(wout[0:64])            # [64,1]
    shared['wout_m'] = np.ascontiguousarray(wout[64:192])          # [128,1]
    shared['bout'] = np.asarray(inputs['b_out'], f32).reshape(1, 1)
    shared['bpf1'] = np.asarray(inputs['b_pf1'], f32).reshape(P, 1)
    shared['bpf2'] = np.asarray(inputs['b_pf2'], f32).reshape(P, 1)
    shared['id32'] = np.eye(32, dtype=f32)
    shared['id128_8'] = _q8(np.eye(P, dtype=f32))
    shared['id64'] = np.eye(64, dtype=f32)

    in_maps = []
    for core in range(N_CORES):
        br = 1 + core // NQ
        q = core % NQ
        bd = bdata[br]
        m = dict(shared)
        m['xt'] = bd['xt']
        m['wg'] = bd['wg']
        m['b8row'] = bd['b8']
        # gather plan
        perm = perms[core]
        flat_idx, smat = _fill_core_gather2(
            cfg, chunks, groups, bd['plans'][q], bd['dinv'], q, perm)
        m['idx'] = _wrap_idx_groups(flat_idx)
        m['smat'] = np.ascontiguousarray(
            smat.transpose(1, 0, 2).reshape(P, n_groups * GRPU * P)).astype(
            ml_dtypes.float8_e4m3)
        # mpool [128, NBLK, B] fp8 (x256); slot j -> physical block perm[j]
        mp = np.zeros((P, cfg.NBLK, cfg.B), f32)
        for j in range(cfg.NBLK):
            nodes = q * cfg.QH + perm[j] * BLK + np.arange(BLK)
            ok = nodes < cfg.N
            gidx = bd['batch'][np.minimum(nodes, cfg.N - 1)]
            val = 256.0 / np.maximum(bd['cnt'][gidx], 1.0) * ok
            mp[np.arange(BLK), j, gidx] = val
        m['mpool'] = _q8(mp.reshape(P, cfg.NBLK * cfg.B))
        # W_pf for this branch  [128, 8*128] f32  (p, k, m)
        wpf = np.asarray(inputs[f'W_pf{br}'], f32)
        m['wpf'] = np.ascontiguousarray(
            wpf.reshape(8, P, P).transpose(1, 0, 2)).reshape(P, 8 * P)
        # branch masks for cc packing
        m['mask1'] = np.full((P, 1), 1.0 if br == 1 else 0.0, f32)
        m['mask2'] = np.full((P, 1), 1.0 if br == 2 else 0.0, f32)
        # masif (this branch only, 8 graphs) laid out [64=(lb,g), C*LBS]
        gs = (core % NQ) * cfg.GPB
        for sfk, name in (('s', 'straight'), ('f', 'flipped')):
            src = np.asarray(inputs[f'mas{br}_{name}'], f32)[gs:gs + cfg.GPB]
            # [g, ch, lb*LBS+l] -> [(lb, g), ch, l]
            r = src.reshape(cfg.GPB, cfg.C, cfg.LB, cfg.LBS).transpose(
                2, 0, 1, 3)
            m[f'mas_{sfk}'] = np.ascontiguousarray(r).reshape(
                64, cfg.C * cfg.LBS)
        # wm128 [10, 8, 128]: this branch's W_m/(2*LW) at rows 64*(br-1)..
        wm = np.zeros((cfg.WPB, cfg.LB, P), f32)
        wsrc = (np.asarray(inputs[f'W_m{br}'], f32) / (2.0 * cfg.LW)).reshape(
            cfg.LB, cfg.WPB, 64)
        wm[:, :, 64 * (br - 1):64 * br] = wsrc.transpose(1, 0, 2)
        m['wm'] = np.ascontiguousarray(wm)
        bm = np.zeros((P, 1), f32)
        bm[64 * (br - 1):64 * br, 0] = np.asarray(inputs[f'b_m{br}'], f32)
        m['bm'] = bm
        gm = np.zeros((P, cfg.B), f32)
        gm[64 * (br - 1):64 * br, gs:gs + cfg.GPB] = 1.0
        m['gmask'] = gm
        for sf, pre in (('s', 'cs'), ('f', 'cf')):
            w = float(np.asarray(inputs[f'{pre}{br}_w'])[0])
            b = float(np.asarray(inputs[f'{pre}{br}_b'])[0])
            m[f'scale_{sf}'] = np.full((64, 1), w / cfg.C, f32)
            m[f'bias_{sf}'] = np.full((64, 1), b, f32)
        in_maps.append(m)
    return meta, in_maps


# ---------------------------------------------------------------- program
def _build(cfg, meta):
    import concourse.bass as bass
    import concourse.bacc as bacc
    import concourse.mybir as mybir
    import concourse.tile as tile

    dt = mybir.dt
    f32 = dt.float32
    fp8 = dt.float8e4
    AF = mybir.ActivationFunctionType
    OP = mybir.AluOpType
    DR = mybir.MatmulPerfMode.DoubleRow if USE_DR else None

    chunks = meta['chunks']
    groups = meta['groups']
    n_groups = meta['n_groups']
    n_ga = meta['n_ga']

    nc = bacc.Bacc("TRN2", target_bir_lowering=False, debug=False,
                   enable_asserts=False, num_devices=N_CORES)

    def din(name, shape, d):
        return nc.dram_tensor(name, list(shape), d, kind="ExternalInput")

    NSLAB = cfg.NPAD // 512
    xt_d = din('xt', (NSLAB, P, cfg.KC2 * 2 * 512), fp8)
    wg_d = din('wg', (P, cfg.KC2 * 2 * cfg.F), fp8)
    b8_d = din('b8row', (1, cfg.F), fp8)
    idx_d = din('idx', (P, n_groups * 64), dt.int16)
    smat_d = din('smat', (P, n_groups * GRPU * P), fp8)
    mpool_d = din('mpool', (P, cfg.NBLK * cfg.B), fp8)
    wpf_d = din('wpf', (P, 8 * P), f32)
    mas_d = {sf: din(f'mas_{sf}', (64, cfg.C * cfg.LBS), f32) for sf in 'sf'}
    wm_d = din('wm', (cfg.WPB, cfg.LB, P), f32)
    bm_d = din('bm', (P, 1), f32)
    gmask_d = din('gmask', (P, cfg.B), f32)
    msc_d = {(sf, kind): din(f'{kind}_{sf}', (64, 1), f32)
             for sf in 'sf' for kind in ('scale', 'bias')}
    wfc1_d = din('wfc1', (P, 2, 256), f32)
    wfc2_d = din('wfc2', (P, 2, 64), f32)
    bfc1_d = din('bfc1', (P, 2, 1), f32)
    bfc2_d = din('bfc2', (64, 1), f32)
    wout_x_d = din('wout_x', (64, 1), f32)
    wout_m_d = din('wout_m', (P, 1), f32)
    bout_d = din('bout', (1, 1), f32)
    bpf1_d = din('bpf1', (P, 1), f32)
    bpf2_d = din('bpf2', (P, 1), f32)
    mask1_d = din('mask1', (P, 1), f32)
    mask2_d = din('mask2', (P, 1), f32)
    id32_d = din('id32', (32, 32), f32)
    id64_d = din('id64', (64, 64), f32)
    id128_d = din('id128_8', (P, P), fp8)

    out_t = nc.dram_tensor('out', [1, cfg.B], f32, kind="ExternalOutput")

    CC = 3 * P * cfg.B          # allreduce payload (f32 elements)

    with tile.TileContext(nc) as tc:
        with tc.tile_pool(name="const", bufs=1) as cst, \
             tc.tile_pool(name="xt", bufs=2) as xtp, \
             tc.tile_pool(name="xwsb", bufs=3) as xwsb, \
             tc.tile_pool(name="gat", bufs=7) as gatp, \
             tc.tile_pool(name="hsb", bufs=2) as hp, \
             tc.tile_pool(name="small", bufs=2) as smp, \
             tc.tile_pool(name="psA", bufs=2, space="PSUM") as psA, \
             tc.tile_pool(name="psBlk", bufs=1, space="PSUM") as psB, \
             tc.tile_pool(name="psPool", bufs=1, space="PSUM") as psP, \
             tc.tile_pool(name="psX", bufs=2, space="PSUM") as psX, \
             tc.tile_pool(name="dram", bufs=1, space="DRAM") as drp:

            def load(pool, src_ap, shape, d, name=None):
                t = pool.tile(list(shape), d, tag=name)
                nc.sync.dma_start(out=t[:], in_=src_ap)
                return t

            # ---------------- xw-critical constants first
            wg_flat = load(cst, wg_d[:, :], (P, cfg.KC2 * 2 * cfg.F), fp8,
                           'wg')
            wg_sb = wg_flat[:].rearrange("p (c i f) -> p c i f", c=cfg.KC2,
                                         i=2)

            # ---------------- xw compute: A half (source rows < SH)
            xwA = drp.tile([cfg.XWA_ROWS, cfg.F], fp8, tag='xwA')
            xwB = drp.tile([cfg.XWB_ROWS, cfg.F], fp8, tag='xwB')

            def xw_slab(sl):
                n0 = sl * 512
                xt_flat = xtp.tile([P, cfg.KC2 * 2 * 512], fp8, tag='xt',
                                   name='xt_t')
                nc.sync.dma_start(out=xt_flat[:], in_=xt_d[sl, :, :])
                xt_t = xt_flat[:].rearrange("p (c i n) -> p c i n", c=cfg.KC2,
                                            i=2)
                for sub in range(4):
                    xw_t = xwsb.tile([P, cfg.F], fp8, tag='xwsb', name='xw_t')
                    for fh in range(2):
                        ps = psA.tile([P, 512], f32, space="PSUM", tag='xwps',
                                      name='xw_ps')
                        for c in range(cfg.KC2):
                            if USE_DR:
                                nc.tensor.matmul(
                                    ps[:],
                                    lhsT=xt_t[:, c, :, sub * P:(sub + 1) * P],
                                    rhs=wg_sb[:, c, :, fh * 512:(fh + 1) * 512],
                                    start=(c == 0), stop=(c == cfg.KC2 - 1),
                                    perf_mode=DR)
                            else:
                                for i in range(2):
                                    nc.tensor.matmul(
                                        ps[:],
                                        lhsT=xt_t[:, c, i,
                                                  sub * P:(sub + 1) * P],
                                        rhs=wg_sb[:, c, i,
                                                  fh * 512:(fh + 1) * 512],
                                        start=(c == 0 and i == 0),
                                        stop=(c == cfg.KC2 - 1 and i == 1))
                        nc.scalar.activation(xw_t[:, fh * 512:(fh + 1) * 512],
                                             ps[:], AF.Identity, scale=0.125)
                    row = n0 + sub * P
                    if row < cfg.SH:
                        nc.sync.dma_start(out=xwA[row:row + P, :], in_=xw_t[:])
                    else:
                        nc.sync.dma_start(
                            out=xwB[row - cfg.SH:row - cfg.SH + P, :],
                            in_=xw_t[:])

            for sl in range(NSLAB // 2):
                xw_slab(sl)

            # ---------------- remaining constants (overlap with xw PE)
            idx_sb = load(cst, idx_d[:, :], (P, n_groups * 64), dt.int16, 'idx')
            smat_sb = load(cst, smat_d[:, :], (P, n_groups * GRPU * P), fp8,
                           'smat')
            mpool_sb = load(cst, mpool_d[:, :], (P, cfg.NBLK * cfg.B), fp8,
                            'mpool')
            wpf_flat = load(cst, wpf_d[:, :], (P, 8 * P), f32, 'wpf')
            wpf_sb = wpf_flat[:].rearrange("p (k m) -> p k m", k=8)
            wm_sb = load(cst, wm_d[:, :, :], (cfg.WPB, cfg.LB, P), f32, 'wm')
            bm_sb = load(cst, bm_d[:, :], (P, 1), f32, 'bm')
            gmask_sb = load(cst, gmask_d[:, :], (P, cfg.B), f32, 'gmask')
            msc_sb = {k: load(cst, v[:, :], (64, 1), f32, f'msc{k}')
                      for k, v in msc_d.items()}
            wfc1_sb = load(cst, wfc1_d[:, :, :], (P, 2, 256), f32, 'wfc1')
            wfc2_sb = load(cst, wfc2_d[:, :, :], (P, 2, 64), f32, 'wfc2')
            bfc1_sb = load(cst, bfc1_d[:, :, :], (P, 2, 1), f32, 'bfc1')
            bfc2_sb = load(cst, bfc2_d[:, :], (64, 1), f32, 'bfc2')
            wout_x_sb = load(cst, wout_x_d[:, :], (64, 1), f32, 'woutx')
            wout_m_sb = load(cst, wout_m_d[:, :], (P, 1), f32, 'woutm')
            bout_sb = load(cst, bout_d[:, :], (1, 1), f32, 'bout')
            bpf1_sb = load(cst, bpf1_d[:, :], (P, 1), f32, 'bpf1')
            bpf2_sb = load(cst, bpf2_d[:, :], (P, 1), f32, 'bpf2')
            mask1_sb = load(cst, mask1_d[:, :], (P, 1), f32, 'mask1')
            mask2_sb = load(cst, mask2_d[:, :], (P, 1), f32, 'mask2')
            id32 = load(cst, id32_d[:, :], (32, 32), f32, 'id32')
            id64 = load(cst, id64_d[:, :], (64, 64), f32, 'id64')
            id128 = load(cst, id128_d[:, :], (P, P), fp8, 'id128')
            b8_sb = load(cst, b8_d[:, :], (1, cfg.F), fp8, 'b8')

            hA = cst.tile([P, cfg.NBLK * cfg.F], fp8, tag='hA')

            # ---------------- masif (one branch, 8 graphs -> [128, B] via PE)
            frag = None
            for sf in 'sf':
                tf = smp.tile([64, cfg.C * cfg.LBS], f32, tag='masload',
                              name='mas_t')
                nc.sync.dma_start(out=tf[:], in_=mas_d[sf][:, :])
                t = tf[:].rearrange("p (c l) -> p c l", c=cfg.C)
                red = smp.tile([64, cfg.LBS], f32, tag='masred')
                nc.vector.tensor_reduce(
                    out=red[:], in_=t.transpose([0, 2, 1]),
                    axis=mybir.AxisListType.X, op=OP.add)
                act = smp.tile([64, cfg.LBS], f32, tag='masact')
                nc.scalar.activation(
                    act[:], red[:], AF.Relu,
                    bias=msc_sb[(sf, 'bias')][:, 0:1],
                    scale=msc_sb[(sf, 'scale')][:, 0:1])
                ws = smp.tile([64, cfg.WPB], f32, tag='masws')
                nc.vector.tensor_reduce(
                    out=ws[:],
                    in_=act[:].rearrange("p (w l) -> p w l", l=cfg.LW),
                    axis=mybir.AxisListType.X, op=OP.add)
                if frag is None:
                    frag = ws
                else:
                    frag2 = smp.tile([64, cfg.WPB], f32, tag='masfrag')
                    nc.vector.tensor_add(out=frag2[:], in0=frag[:], in1=ws[:])
                    frag = frag2
            ps_t = psX.tile([cfg.WPB, 64], f32, space="PSUM", tag='aux')
            nc.tensor.transpose(out=ps_t[:], in_=frag[:], identity=id64[:])
            fragT = smp.tile([cfg.WPB, 64], f32, tag='masfragT')
            nc.scalar.activation(fragT[:], ps_t[:], AF.Identity)
            m_ps = psX.tile([P, cfg.GPB], f32, space="PSUM", tag='aux')
            for lb in range(cfg.LB):
                nc.tensor.matmul(
                    m_ps[:], lhsT=wm_sb[:, lb, :],
                    rhs=fragT[:, lb * cfg.GPB:(lb + 1) * cfg.GPB],
                    start=(lb == 0), stop=(lb == cfg.LB - 1))
            m_fm = smp.tile([P, cfg.GPB], f32, tag='masfm')
            nc.scalar.activation(m_fm[:], m_ps[:], AF.Identity,
                                 bias=bm_sb[:, 0:1])
            t_mas = cst.tile([P, cfg.B], f32, tag='tmas')
            nc.vector.tensor_tensor(
                out=t_mas[:].rearrange("p (s g) -> p s g", g=cfg.GPB),
                in0=m_fm[:, None, :].to_broadcast([P, NQ, cfg.GPB]),
                in1=gmask_sb[:, :].rearrange("p (s g) -> p s g", g=cfg.GPB),
                op=OP.mult)

            # bias row + zero row of xwA, then B-half slabs
            zrow = smp.tile([1, cfg.F], fp8, tag='zrow')
            nc.vector.memset(zrow[:], 0.0)
            nc.sync.dma_start(out=xwA[cfg.SH:cfg.SH + 1, :], in_=b8_sb[:])
            nc.sync.dma_start(out=xwA[cfg.SH + 1:cfg.SH + 2, :], in_=zrow[:])

            for sl in range(NSLAB // 2, NSLAB):
                xw_slab(sl)

            # ---------------- gather + scatter + pool
            pool_ps = [psP.tile([cfg.B, 512], f32, space="PSUM",
                                name=f'poolps{fh}') for fh in range(2)]
            blk_ps = {}
            gat_tiles = {}
            # per-group gathers; chunks reference their group's tile
            ch_by_grp = {}
            for ch in chunks:
                ch_by_grp.setdefault(ch['grp'], []).append(ch)

            pooled_n = [0]

            def finish_block(j, ps_pair, phase):
                if phase == 0:
                    # stage A partial (8x scale) into hA as fp8
                    for fh in range(2):
                        nc.scalar.activation(
                            hA[:, j * cfg.F + fh * 512:
                               j * cfg.F + (fh + 1) * 512],
                            ps_pair[fh][:], AF.Identity)
                else:
                    h_t = hp.tile([P, cfg.F], fp8, tag='h')
                    for fh in range(2):
                        nc.scalar.activation(
                            h_t[:, fh * 512:(fh + 1) * 512], ps_pair[fh][:],
                            AF.Lrelu, scale=0.125, alpha=0.01)
                    for fh in range(2):
                        nc.tensor.matmul(
                            pool_ps[fh][:],
                            lhsT=mpool_sb[:, j * cfg.B:(j + 1) * cfg.B],
                            rhs=h_t[:, fh * 512:(fh + 1) * 512],
                            start=(pooled_n[0] == 0),
                            stop=(pooled_n[0] == cfg.NBLK - 1))
                    pooled_n[0] += 1

            for g in range(n_groups):
                hf = groups[g][0]
                src = xwA if hf == 0 else xwB
                gat = gatp.tile([P, GRPU, cfg.F], fp8, tag='gat')
                nc.gpsimd.dma_gather(
                    out_ap=gat[:], in_ap=src[:, :],
                    idxs_ap=idx_sb[:, g * 64:(g + 1) * 64],
                    num_idxs=GRPU * P, num_idxs_reg=GRPU * P,
                    elem_size=cfg.F)
                for ch in ch_by_grp[g]:
                    j, u0 = ch['j'], ch['u0']
                    if ch['first']:
                        pair = [psB.tile([P, 512], f32, space="PSUM",
                                         name=f'blkps{fh}') for fh in range(2)]
                        blk_ps[(j, hf)] = pair
                        if hf == 1:
                            for fh in range(2):
                                nc.tensor.matmul(
                                    pair[fh][:], lhsT=id128[:],
                                    rhs=hA[:, j * cfg.F + fh * 512:
                                           j * cfg.F + (fh + 1) * 512],
                                    start=True, stop=False)
                    pair = blk_ps[(j, hf)]
                    sm0 = (g * GRPU + u0) * P
                    st = ch['first'] and hf == 0
                    sp = ch['last']
                    for fh in range(2):
                        if ch['units'] == 2 and USE_DR:
                            nc.tensor.matmul(
                                pair[fh][:],
                                lhsT=smat_sb[:, sm0:sm0 + 2 * P].rearrange(
                                    "p (i d) -> p i d", i=2),
                                rhs=gat[:, u0:u0 + 2,
                                        fh * 512:(fh + 1) * 512],
                                start=st, stop=sp, perf_mode=DR)
                        else:
                            for i in range(ch['units']):
                                nc.tensor.matmul(
                                    pair[fh][:],
                                    lhsT=smat_sb[:, sm0 + i * P:
                                                 sm0 + (i + 1) * P],
                                    rhs=gat[:, u0 + i,
                                            fh * 512:(fh + 1) * 512],
                                    start=(st and i == 0),
                                    stop=(sp and i == ch['units'] - 1))
                for ch in ch_by_grp[g]:
                    if ch['last']:
                        finish_block(ch['j'], blk_ps.pop((ch['j'], hf)), hf)

            # ---------------- pooled -> x1 partial
            pooled_sb = smp.tile([cfg.B, cfg.F], f32, tag='pooled')
            for fh in range(2):
                nc.scalar.activation(pooled_sb[:, fh * 512:(fh + 1) * 512],
                                     pool_ps[fh][:], AF.Identity,
                                     scale=float(2.0 ** -8))
            pfm = smp.tile([P, 8, cfg.B], f32, tag='pfm')
            for k in range(8):
                tps = psX.tile([P, cfg.B], f32, space="PSUM", tag='aux')
                nc.tensor.transpose(
                    out=tps[:], in_=pooled_sb[:, k * P:(k + 1) * P],
                    identity=id32[:])
                nc.scalar.activation(pfm[:, k, :], tps[:], AF.Identity)
            xps = psX.tile([P, cfg.B], f32, space="PSUM", tag='aux')
            for k in range(8):
                nc.tensor.matmul(xps[:], lhsT=wpf_sb[:, k, :],
                                 rhs=pfm[:, k, :],
                                 start=(k == 0), stop=(k == 7))
            x1p = smp.tile([P, cfg.B], f32, tag='x1p')
            nc.scalar.activation(x1p[:], xps[:], AF.Identity)

            # ---------------- cc packing + allreduce
            t_x1 = smp.tile([P, cfg.B], f32, tag='tx1')
            t_x2 = smp.tile([P, cfg.B], f32, tag='tx2')
            nc.scalar.activation(t_x1[:], x1p[:], AF.Identity,
                                 scale=mask1_sb[:, 0:1])
            nc.scalar.activation(t_x2[:], x1p[:], AF.Identity,
                                 scale=mask2_sb[:, 0:1])
            bounce_in = drp.tile([CC], f32, tag='ccin')
            bounce_out = drp.tile([N_CORES * CC], f32, tag='ccout')
            seg = P * cfg.B
            for i, t in enumerate((t_x1, t_x2, t_mas)):
                nc.sync.dma_start(
                    out=bounce_in[i * seg:(i + 1) * seg].rearrange(
                        "(p f) -> p f", f=cfg.B),
                    in_=t[:])
            nc.gpsimd.collective_compute(
                "AllGather", OP.bypass,
                replica_groups=[list(range(N_CORES))],
                ins=[bounce_in[:].opt()], outs=[bounce_out[:].opt()])
            gath_v = bounce_out[:].rearrange(
                "(r t p f) -> t p r f", r=N_CORES, t=3, p=P)

            def cc_sum(ti, name):
                raw = smp.tile([P, N_CORES, cfg.B], f32, tag='ccraw',
                               name=f'raw{name}')
                nc.sync.dma_start(out=raw[:], in_=gath_v[ti])
                red = smp.tile([P, cfg.B], f32, tag=f'ccred{name}',
                               name=f'red{name}')
                nc.vector.tensor_reduce(
                    out=red[:], in_=raw[:].transpose([0, 2, 1]),
                    axis=mybir.AxisListType.X, op=OP.add)
                return red

            x12 = {}
            for brr, bpf in ((1, bpf1_sb), (2, bpf2_sb)):
                xs = cc_sum(brr - 1, f'x{brr}')
                nc.scalar.activation(xs[:], xs[:], AF.Lrelu,
                                     bias=bpf[:, 0:1], alpha=0.01)
                x12[brr] = xs
            masif_rb = cc_sum(2, 'mas')

            # ---------------- head
            xc1 = {}
            for mh in range(2):
                cps = psX.tile([P, cfg.B], f32, space="PSUM", tag='aux')
                for k2 in range(2):
                    nc.tensor.matmul(
                        cps[:], lhsT=wfc1_sb[:, k2, mh * P:(mh + 1) * P],
                        rhs=x12[k2 + 1][:], start=(k2 == 0), stop=(k2 == 1))
                xcs = smp.tile([P, cfg.B], f32, tag=f'xc{mh}')
                nc.scalar.activation(xcs[:], cps[:], AF.Lrelu,
                                     bias=bfc1_sb[:, mh, 0:1], alpha=0.01)
                xc1[mh] = xcs
            c2ps = psX.tile([64, cfg.B], f32, space="PSUM", tag='aux')
            for k2 in range(2):
                nc.tensor.matmul(c2ps[:], lhsT=wfc2_sb[:, k2, :],
                                 rhs=xc1[k2][:], start=(k2 == 0),
                                 stop=(k2 == 1))
            xc = smp.tile([64, cfg.B], f32, tag='xcf')
            nc.scalar.activation(xc[:], c2ps[:], AF.Lrelu,
                                 bias=bfc2_sb[:, 0:1], alpha=0.01)

            ops = psX.tile([1, cfg.B], f32, space="PSUM", tag='aux')
            nc.tensor.matmul(ops[:], lhsT=wout_x_sb[:], rhs=xc[:],
                             start=True, stop=False)
            nc.tensor.matmul(ops[:], lhsT=wout_m_sb[:], rhs=masif_rb[:],
                             start=False, stop=True)
            res = smp.tile([1, cfg.B], f32, tag='res')
            nc.scalar.activation(res[:], ops[:], AF.Sigmoid,
                                 bias=bout_sb[:, 0:1])
            nc.sync.dma_start(out=out_t[:, :], in_=res[:])

    nc.compile()
    return nc


# ---------------------------------------------------------------- entry
_CACHE = {}


def _run(inputs, cfg, trace=False, tmpdir=None):
    from concourse import bass_utils
    meta, in_maps = _preprocess(inputs, cfg)
    key = tuple((c['hf'], c['j'], c['units'], c['u0'], c['grp'],
                 c['first'], c['last']) for c in meta['chunks'])
    if key not in _CACHE:
        _CACHE.clear()
        _CACHE[key] = _build(cfg, meta)
    nc = _CACHE[key]
    res = bass_utils.run_bass_kernel_spmd(
        nc, in_maps, core_ids=list(range(N_CORES)), trace=trace, tmpdir=tmpdir)
    out = np.asarray(res.results[0]['out'], np.float32).reshape(cfg.B, 1)
    return out, res


def kernel(**inputs) -> np.ndarray:
    cfg = _Cfg()
    out, _ = _run(inputs, cfg)
    return out


# revision 24
# speedup vs baseline: 1.0415x; 1.0415x over previous
"""Trainium2 Bass kernel for nn_GCNN_87668872446200.

Branch-split design over 8 NeuronCores: cores 0-3 run protein branch 1,
cores 4-7 run branch 2.  Within a branch group each core owns a quarter of
the destination nodes and the full F=1024 feature dim.

Per core (fp8 e4m3 data paths, DoubleRow fp8 matmuls):
  - xw' = 8*(x*dinv_row) @ (W*64) / 8   computed on PE in two source-halves,
    written to HBM as two tensors (xwA rows <5120 + bias row, xwB rest)
  - symmetric norm is separated: h = Dinv A Dinv xw + b realized as
    S-matmul with S[e,d] = dinv[d] (bias via a virtual edge to a bias row)
  - dma_gather pulls 1KB fp8 rows per edge; source-half split lets the
    Q7 descriptor emission of half A overlap the xw compute of half B
  - phase A partial sums staged in SBUF (fp8), injected into phase B PSUM
    via an identity matmul; one ACT pass does lrelu(psum/8)
  - per-graph mean-pool as PE matmul (mpool*256 fp8), W_pf applied locally
  - masif branch (8 graphs/core, this core's branch only)
  - one small AllReduce ([3,128,32] f32 = 48KB) + replicated dense head

All 8 cores run ONE identical program; per-core variation is in input data.
"""
import numpy as np

# ---------------------------------------------------------------- constants
N_CORES = 8
P = 128
BLK = 128           # dest nodes per block
NQ = 4              # dest quarters per branch group
GRPU = 8            # 128-idx units per gather call (1024 idxs)

N_NODES, N_EDGES, F_DIM, B_GRAPHS, L_MAS, C_MAS = 10000, 80000, 1024, 32, 800, 16

USE_DR = True       # DoubleRow fp8 matmuls


class _Cfg:
    def __init__(self, n=N_NODES, e=N_EDGES, f=F_DIM, b=B_GRAPHS,
                 l=L_MAS, c=C_MAS):
        self.N, self.E, self.F, self.B, self.L, self.C = n, e, f, b, l, c
        self.NPAD = ((n + 511) // 512) * 512          # 10240
        self.QH = self.NPAD // NQ                     # 2560 dests per core
        self.NBLK = self.QH // BLK                    # 20 blocks
        self.SH = self.NPAD // 2                      # 5120 source-half split
        self.KC2 = f // 256                           # 4 k-pairs
        self.GPB = b // 4                             # 8 graphs per core
        self.LW = l // 80                             # 10
        self.LB = 8                                   # l-blocks
        self.LBS = l // self.LB                       # 100
        self.WPB = self.LBS // self.LW                # 10
        # xwA holds source rows 0..SH-1 plus bias row (SH) and zero row (SH+1)
        self.XWA_ROWS = self.SH + P                   # 5248
        self.XWB_ROWS = self.NPAD - self.SH           # 5120 (tail rows zero)


def _q8(x):
    import ml_dtypes
    return np.clip(np.asarray(x, np.float32), -240.0, 240.0).astype(
        ml_dtypes.float8_e4m3)


# ---------------------------------------------------------------- host prep
def _edge_plan_core(cfg, edge_index, q):
    """Edges targeting quarter q, split per (block, source-half), sorted.
    Returns dict (j, hf) -> (rows, dests, counts)."""
    row = np.asarray(edge_index[0]).astype(np.int64)
    col = np.asarray(edge_index[1]).astype(np.int64)
    loops = np.arange(cfg.N, dtype=np.int64)
    rows = np.concatenate([row, loops])
    cols = np.concatenate([col, loops])
    lo, hi = q * cfg.QH, (q + 1) * cfg.QH
    sel = (cols >= lo) & (cols < hi)
    r, c = rows[sel], cols[sel] - lo
    out = {}
    for j in range(cfg.NBLK):
        bsel = (c >= j * BLK) & (c < (j + 1) * BLK)
        rj, cj = r[bsel], c[bsel] - j * BLK
        for hf in range(2):
            hsel = (rj < cfg.SH) if hf == 0 else (rj >= cfg.SH)
            out[(j, hf)] = (rj[hsel], cj[hsel])
    return out


def _shared_schedule(cfg, plans):
    """Shared chunk schedule (max over the 8 per-core plans).

    Returns chunks: list of dicts with keys
      hf, j, units (1 or 2), u0 (unit offset in group), grp (group index),
      first (starts block), last (ends block's half... block completion is
      tracked at (j,hf==1,last) for B and (j,hf==0,last) for A)
    and n_groups_a / n_groups_b.
    """
    # per-plan unit needs and slot permutation (sort blocks big->small so the
    # max-over-cores slot schedule aligns; mpool/idx/smat are per-core data)
    def units_of(p, j, hf):
        e = len(p[(j, hf)][0]) + (1 if hf == 0 else 0)
        return max(1, (e + P - 1) // P)

    perms = []
    for p in plans:
        tot = [units_of(p, j, 0) + units_of(p, j, 1) for j in range(cfg.NBLK)]
        perms.append(list(np.argsort(-np.asarray(tot), kind='stable')))

    slot_need = {}
    for s in range(cfg.NBLK):
        for hf in range(2):
            slot_need[(s, hf)] = max(
                units_of(p, perm[s], hf) for p, perm in zip(plans, perms))

    chunks = []
    groups = []                                   # list of [hf, nunits]
    for hf in range(2):
        space = 0                                 # force new group per half
        for s in range(cfg.NBLK):
            left = slot_need[(s, hf)]
            first = True
            while left:
                if space == 0:
                    groups.append([hf, 0])
                    space = GRPU
                sz = 2 if (left >= 2 and space >= 2) else 1
                chunks.append(dict(hf=hf, j=s, units=sz, u0=GRPU - space,
                                   grp=len(groups) - 1,
                                   first=first, last=(left - sz == 0)))
                space -= sz
                left -= sz
                first = False
                groups[-1][1] = GRPU - space
    return chunks, groups, perms


def _fill_core_gather2(cfg, chunks, groups, plan, dinv, q, perm):
    """Per-core idx + smat content for the shared schedule (slot j maps to
    physical block perm[j] for this core)."""
    n_groups = len(groups)
    flat_idx = np.zeros((n_groups, GRPU * P), np.int64)
    smat = np.zeros((n_groups, P, GRPU * P), np.float32)
    dinv8 = _q8(dinv).astype(np.float32)
    consumed = {}
    for ch in chunks:
        sl, hf, g, u0 = ch['j'], ch['hf'], ch['grp'], ch['u0']
        j = perm[sl]
        r, c = plan[(j, hf)]
        off = consumed.get((j, hf), 0)
        cap = ch['units'] * P
        base = u0 * P
        pad_idx = cfg.SH + 1 if hf == 0 else cfg.XWB_ROWS - 1
        flat_idx[g, base:base + cap] = pad_idx
        s = 0
        if hf == 0 and ch['first']:
            flat_idx[g, base] = cfg.SH            # bias row at slot 0
            smat[g, 0, base:base + P] = 1.0
            s = 1
        take = min(len(r) - off, cap - s)
        if take > 0:
            rr = r[off:off + take]
            cc = c[off:off + take]
            if hf == 1:
                rr = rr - cfg.SH
            slots = np.arange(s, s + take) + base
            up = slots // P
            pp = slots % P
            flat_idx[g, slots] = rr
            # dest scale dinv[global dest] ; global dest = q*QH + j*BLK + cc
            gd = q * cfg.QH + j * BLK + cc
            vals = dinv8[np.minimum(gd, cfg.N - 1)] * (gd < cfg.N)
            smat[g, pp, up * P + cc] = vals
        consumed[(j, hf)] = off + take
    for (j, hf), off in consumed.items():
        assert off == len(plan[(j, hf)][0]), (j, hf, off, len(plan[(j, hf)][0]))
    return flat_idx, smat


def _wrap_idx_groups(flat_idx):
    """[G, 1024] -> [128, G*64] int16 (16-part wrap, 8x replicated)."""
    g, n = flat_idx.shape
    w = flat_idx.reshape(g, n // 16, 16).transpose(2, 0, 1).reshape(16, -1)
    return np.tile(w, (8, 1)).astype(np.int16)


def _preprocess(inputs, cfg):
    import ml_dtypes
    bf16 = ml_dtypes.bfloat16
    f32 = np.float32

    # --- per-branch shared data
    bdata = {}
    for br in (1, 2):
        x = np.asarray(inputs[f'pro{br}_x'], f32)
        ei = np.asarray(inputs[f'pro{br}_edge_index'])
        batch = np.asarray(inputs[f'pro{br}_batch']).astype(np.int64)
        row = ei[0].astype(np.int64)
        col = ei[1].astype(np.int64)
        deg = np.bincount(np.concatenate([col, np.arange(cfg.N)]),
                          minlength=cfg.N).astype(np.float64)
        dinv = (1.0 / np.sqrt(deg)).astype(f32)
        # xt_dr [NSLAB, 128, KC2*2*512] fp8 of (x*dinv_row)^T, slab-major
        xp = x * dinv[:, None]
        xpT = np.zeros((cfg.F, cfg.NPAD), f32)
        xpT[:, :cfg.N] = xp.T
        nslab = cfg.NPAD // 512
        xt_dr = np.ascontiguousarray(
            xpT.reshape(cfg.KC2, 2, P, nslab, 512).transpose(3, 2, 0, 1, 4)
        ).reshape(nslab, P, cfg.KC2 * 2 * 512)
        # wg_dr [128, KC2*2*F] fp8  (p, c, i, f)
        W = np.asarray(inputs[f'W_g{br}'], f32) * 64.0
        wg_dr = np.ascontiguousarray(
            W.reshape(cfg.KC2, 2, P, cfg.F).transpose(2, 0, 1, 3)
        ).reshape(P, cfg.KC2 * 2 * cfg.F)
        b8 = np.asarray(inputs[f'b_g{br}'], f32) * 8.0
        cnt = np.bincount(batch, minlength=cfg.B).astype(f32)
        plans = [_edge_plan_core(cfg, ei, q) for q in range(NQ)]
        bdata[br] = dict(xt=_q8(xt_dr), wg=_q8(wg_dr), b8=_q8(b8[None, :]),
                         dinv=dinv, batch=batch, cnt=cnt, plans=plans)

    # --- shared chunk schedule (max over all 8 core plans, slot-permuted)
    all_plans = bdata[1]['plans'] + bdata[2]['plans']
    chunks, groups, perms = _shared_schedule(cfg, all_plans)
    n_groups = len(groups)
    n_ga = sum(1 for g in groups if g[0] == 0)

    meta = dict(chunks=chunks, groups=groups, n_groups=n_groups, n_ga=n_ga)

    # --- head weights (shared across cores)
    shared = {}
    shared['wfc1'] = np.ascontiguousarray(
        np.asarray(inputs['W_fc1'], f32).reshape(2, P, 256).transpose(1, 0, 2))
    shared['wfc2'] = np.ascontiguousarray(
        np.asarray(inputs['W_fc2'], f32).reshape(2, P, 64).transpose(1, 0, 2))
    shared['bfc1'] = np.ascontiguousarray(
        np.asarray(inputs['b_fc1'], f32).reshape(2, P, 1).transpose(1, 0, 2))
    shared['bfc2'] = np.asarray(inputs['b_fc2'], f32).reshape(64, 1)
    wout = np.asarray(inputs['W_out'], f32)
    shared['wout_x'] = np.ascontiguousarray(wout[0:64])            # [64,1]
    shared['wout_m'] = np.ascontiguousarray(wout[64:192])          # [128,1]
    shared['bout'] = np.asarray(inputs['b_out'], f32).reshape(1, 1)
    shared['bpf1'] = np.asarray(inputs['b_pf1'], f32).reshape(P, 1)
    shared['bpf2'] = np.asarray(inputs['b_pf2'], f32).reshape(P, 1)
    shared['id32'] = np.eye(32, dtype=f32)
    shared['id128_8'] = _q8(np.eye(P, dtype=f32))
    shared['id64'] = np.eye(64, dtype=f32)

    in_maps = []
    for core in range(N_CORES):
        br = 1 + core // NQ
        q = core % NQ
        bd = bdata[br]
        m = dict(shared)
        m['xt'] = bd['xt']
        m['wg'] = bd['wg']
        m['b8row'] = bd['b8']
        # gather plan
        perm = perms[core]
        flat_idx, smat = _fill_core_gather2(
            cfg, chunks, groups, bd['plans'][q], bd['dinv'], q, perm)
        m['idx'] = _wrap_idx_groups(flat_idx)
        m['smat'] = np.ascontiguousarray(
            smat.transpose(1, 0, 2).reshape(P, n_groups * GRPU * P)).astype(
            ml_dtypes.float8_e4m3)
        # mpool [128, NBLK, B] fp8 (x256); slot j -> physical block perm[j]
        mp = np.zeros((P, cfg.NBLK, cfg.B), f32)
        for j in range(cfg.NBLK):
            nodes = q * cfg.QH + perm[j] * BLK + np.arange(BLK)
            ok = nodes < cfg.N
            gidx = bd['batch'][np.minimum(nodes, cfg.N - 1)]
            val = 256.0 / np.maximum(bd['cnt'][gidx], 1.0) * ok
            mp[np.arange(BLK), j, gidx] = val
        m['mpool'] = _q8(mp.reshape(P, cfg.NBLK * cfg.B))
        # W_pf for this branch  [128, 8*128] f32  (p, k, m)
        wpf = np.asarray(inputs[f'W_pf{br}'], f32)
        m['wpf'] = np.ascontiguousarray(
            wpf.reshape(8, P, P).transpose(1, 0, 2)).reshape(P, 8 * P)
        # branch masks for cc packing
        m['mask1'] = np.full((P, 1), 1.0 if br == 1 else 0.0, f32)
        m['mask2'] = np.full((P, 1), 1.0 if br == 2 else 0.0, f32)
        # masif (this branch only, 8 graphs) laid out [64=(lb,g), C*LBS]
        gs = (core % NQ) * cfg.GPB
        for sfk, name in (('s', 'straight'), ('f', 'flipped')):
            src = np.asarray(inputs[f'mas{br}_{name}'], f32)[gs:gs + cfg.GPB]
            # [g, ch, lb*LBS+l] -> [(lb, g), ch, l]
            r = src.reshape(cfg.GPB, cfg.C, cfg.LB, cfg.LBS).transpose(
                2, 0, 1, 3)
            m[f'mas_{sfk}'] = np.ascontiguousarray(r).reshape(
                64, cfg.C * cfg.LBS)
        # wm128 [10, 8, 128]: this branch's W_m/(2*LW) at rows 64*(br-1)..
        wm = np.zeros((cfg.WPB, cfg.LB, P), f32)
        wsrc = (np.asarray(inputs[f'W_m{br}'], f32) / (2.0 * cfg.LW)).reshape(
            cfg.LB, cfg.WPB, 64)
        wm[:, :, 64 * (br - 1):64 * br] = wsrc.transpose(1, 0, 2)
        m['wm'] = np.ascontiguousarray(wm)
        bm = np.zeros((P, 1), f32)
        bm[64 * (br - 1):64 * br, 0] = np.asarray(inputs[f'b_m{br}'], f32)
        m['bm'] = bm
        gm = np.zeros((P, cfg.B), f32)
        gm[64 * (br - 1):64 * br, gs:gs + cfg.GPB] = 1.0
        m['gmask'] = gm
        for sf, pre in (('s', 'cs'), ('f', 'cf')):
            w = float(np.asarray(inputs[f'{pre}{br}_w'])[0])
            b = float(np.asarray(inputs[f'{pre}{br}_b'])[0])
            m[f'scale_{sf}'] = np.full((64, 1), w / cfg.C, f32)
            m[f'bias_{sf}'] = np.full((64, 1), b, f32)
        in_maps.append(m)
    return meta, in_maps


# ---------------------------------------------------------------- program
def _build(cfg, meta):
    import concourse.bass as bass
    import concourse.bacc as bacc
    import concourse.mybir as mybir
    import concourse.tile as tile

    dt = mybir.dt
    f32 = dt.float32
    fp8 = dt.float8e4
    AF = mybir.ActivationFunctionType
    OP = mybir.AluOpType
    DR = mybir.MatmulPerfMode.DoubleRow if USE_DR else None

    chunks = meta['chunks']
    groups = meta['groups']
    n_groups = meta['n_groups']
    n_ga = meta['n_ga']

    nc = bacc.Bacc("TRN2", target_bir_lowering=False, debug=False,
                   enable_asserts=False, num_devices=N_CORES)

    def din(name, shape, d):
        return nc.dram_tensor(name, list(shape), d, kind="ExternalInput")

    NSLAB = cfg.NPAD // 512
    xt_d = din('xt', (NSLAB, P, cfg.KC2 * 2 * 512), fp8)
    wg_d = din('wg', (P, cfg.KC2 * 2 * cfg.F), fp8)
    b8_d = din('b8row', (1, cfg.F), fp8)
    idx_d = din('idx', (P, n_groups * 64), dt.int16)
    smat_d = din('smat', (P, n_groups * GRPU * P), fp8)
    mpool_d = din('mpool', (P, cfg.NBLK * cfg.B), fp8)
    wpf_d = din('wpf', (P, 8 * P), f32)
    mas_d = {sf: din(f'mas_{sf}', (64, cfg.C * cfg.LBS), f32) for sf in 'sf'}
    wm_d = din('wm', (cfg.WPB, cfg.LB, P), f32)
    bm_d = din('bm', (P, 1), f32)
    gmask_d = din('gmask', (P, cfg.B), f32)
    msc_d = {(sf, kind): din(f'{kind}_{sf}', (64, 1), f32)
             for sf in 'sf' for kind in ('scale', 'bias')}
    wfc1_d = din('wfc1', (P, 2, 256), f32)
    wfc2_d = din('wfc2', (P, 2, 64), f32)
    bfc1_d = din('bfc1', (P, 2, 1), f32)
    bfc2_d = din('bfc2', (64, 1), f32)
    wout_x_d = din('wout_x', (64, 1), f32)
    wout_m_d = din('wout_m', (P, 1), f32)
    bout_d = din('bout', (1, 1), f32)
    bpf1_d = din('bpf1', (P, 1), f32)
    bpf2_d = din('bpf2', (P, 1), f32)
    mask1_d = din('mask1', (P, 1), f32)
    mask2_d = din('mask2', (P, 1), f32)
    id32_d = din('id32', (32, 32), f32)
    id64_d = din('id64', (64, 64), f32)
    id128_d = din('id128_8', (P, P), fp8)

    out_t = nc.dram_tensor('out', [1, cfg.B], f32, kind="ExternalOutput")

    CC = 3 * P * cfg.B          # allreduce payload (f32 elements)

    with tile.TileContext(nc) as tc:
        with tc.tile_pool(name="const", bufs=1) as cst, \
             tc.tile_pool(name="xt", bufs=2) as xtp, \
             tc.tile_pool(name="xwsb", bufs=3) as xwsb, \
             tc.tile_pool(name="gat", bufs=5) as gatp, \
             tc.tile_pool(name="hsb", bufs=2) as hp, \
             tc.tile_pool(name="small", bufs=2) as smp, \
             tc.tile_pool(name="psA", bufs=2, space="PSUM") as psA, \
             tc.tile_pool(name="psBlk", bufs=1, space="PSUM") as psB, \
             tc.tile_pool(name="psPool", bufs=1, space="PSUM") as psP, \
             tc.tile_pool(name="psX", bufs=2, space="PSUM") as psX, \
             tc.tile_pool(name="dram", bufs=1, space="DRAM") as drp:

            def load(pool, src_ap, shape, d, name=None):
                t = pool.tile(list(shape), d, tag=name)
                nc.sync.dma_start(out=t[:], in_=src_ap)
                return t

            # ---------------- xw-critical constants first
            wg_flat = load(cst, wg_d[:, :], (P, cfg.KC2 * 2 * cfg.F), fp8,
                           'wg')
            wg_sb = wg_flat[:].rearrange("p (c i f) -> p c i f", c=cfg.KC2,
                                         i=2)

            # ---------------- xw compute: A half (source rows < SH)
            xwA = drp.tile([cfg.XWA_ROWS, cfg.F], fp8, tag='xwA')
            xwB = drp.tile([cfg.XWB_ROWS, cfg.F], fp8, tag='xwB')

            def xw_slab(sl):
                n0 = sl * 512
                xt_flat = xtp.tile([P, cfg.KC2 * 2 * 512], fp8, tag='xt',
                                   name='xt_t')
                nc.sync.dma_start(out=xt_flat[:], in_=xt_d[sl, :, :])
                xt_t = xt_flat[:].rearrange("p (c i n) -> p c i n", c=cfg.KC2,
                                            i=2)
                for sub in range(4):
                    xw_t = xwsb.tile([P, cfg.F], fp8, tag='xwsb', name='xw_t')
                    for fh in range(2):
                        ps = psA.tile([P, 512], f32, space="PSUM", tag='xwps',
                                      name='xw_ps')
                        for c in range(cfg.KC2):
                            if USE_DR:
                                nc.tensor.matmul(
                                    ps[:],
                                    lhsT=xt_t[:, c, :, sub * P:(sub + 1) * P],
                                    rhs=wg_sb[:, c, :, fh * 512:(fh + 1) * 512],
                                    start=(c == 0), stop=(c == cfg.KC2 - 1),
                                    perf_mode=DR)
                            else:
                                for i in range(2):
                                    nc.tensor.matmul(
                                        ps[:],
                                        lhsT=xt_t[:, c, i,
                                                  sub * P:(sub + 1) * P],
                                        rhs=wg_sb[:, c, i,
                                                  fh * 512:(fh + 1) * 512],
                                        start=(c == 0 and i == 0),
                                        stop=(c == cfg.KC2 - 1 and i == 1))
                        nc.scalar.activation(xw_t[:, fh * 512:(fh + 1) * 512],
                                             ps[:], AF.Identity, scale=0.125)
                    row = n0 + sub * P
                    if row < cfg.SH:
                        nc.sync.dma_start(out=xwA[row:row + P, :], in_=xw_t[:])
                    else:
                        nc.sync.dma_start(
                            out=xwB[row - cfg.SH:row - cfg.SH + P, :],
                            in_=xw_t[:])

            for sl in range(NSLAB // 2):
                xw_slab(sl)

            # ---------------- remaining constants (overlap with xw PE)
            idx_sb = load(cst, idx_d[:, :], (P, n_groups * 64), dt.int16, 'idx')
            smat_sb = load(cst, smat_d[:, :], (P, n_groups * GRPU * P), fp8,
                           'smat')
            mpool_sb = load(cst, mpool_d[:, :], (P, cfg.NBLK * cfg.B), fp8,
                            'mpool')
            wpf_flat = load(cst, wpf_d[:, :], (P, 8 * P), f32, 'wpf')
            wpf_sb = wpf_flat[:].rearrange("p (k m) -> p k m", k=8)
            wm_sb = load(cst, wm_d[:, :, :], (cfg.WPB, cfg.LB, P), f32, 'wm')
            bm_sb = load(cst, bm_d[:, :], (P, 1), f32, 'bm')
            gmask_sb = load(cst, gmask_d[:, :], (P, cfg.B), f32, 'gmask')
            msc_sb = {k: load(cst, v[:, :], (64, 1), f32, f'msc{k}')
                      for k, v in msc_d.items()}
            wfc1_sb = load(cst, wfc1_d[:, :, :], (P, 2, 256), f32, 'wfc1')
            wfc2_sb = load(cst, wfc2_d[:, :, :], (P, 2, 64), f32, 'wfc2')
            bfc1_sb = load(cst, bfc1_d[:, :, :], (P, 2, 1), f32, 'bfc1')
            bfc2_sb = load(cst, bfc2_d[:, :], (64, 1), f32, 'bfc2')
            wout_x_sb = load(cst, wout_x_d[:, :], (64, 1), f32, 'woutx')
            wout_m_sb = load(cst, wout_m_d[:, :], (P, 1), f32, 'woutm')
            bout_sb = load(cst, bout_d[:, :], (1, 1), f32, 'bout')
            bpf1_sb = load(cst, bpf1_d[:, :], (P, 1), f32, 'bpf1')
            bpf2_sb = load(cst, bpf2_d[:, :], (P, 1), f32, 'bpf2')
            mask1_sb = load(cst, mask1_d[:, :], (P, 1), f32, 'mask1')
            mask2_sb = load(cst, mask2_d[:, :], (P, 1), f32, 'mask2')
            id32 = load(cst, id32_d[:, :], (32, 32), f32, 'id32')
            id64 = load(cst, id64_d[:, :], (64, 64), f32, 'id64')
            id128 = load(cst, id128_d[:, :], (P, P), fp8, 'id128')
            b8_sb = load(cst, b8_d[:, :], (1, cfg.F), fp8, 'b8')

            hA = cst.tile([P, cfg.NBLK * cfg.F], fp8, tag='hA')

            # ---------------- masif (one branch, 8 graphs -> [128, B] via PE)
            frag = None
            for sf in 'sf':
                tf = smp.tile([64, cfg.C * cfg.LBS], f32, tag='masload',
                              name='mas_t')
                nc.sync.dma_start(out=tf[:], in_=mas_d[sf][:, :])
                t = tf[:].rearrange("p (c l) -> p c l", c=cfg.C)
                red = smp.tile([64, cfg.LBS], f32, tag='masred')
                nc.vector.tensor_reduce(
                    out=red[:], in_=t.transpose([0, 2, 1]),
                    axis=mybir.AxisListType.X, op=OP.add)
                act = smp.tile([64, cfg.LBS], f32, tag='masact')
                nc.scalar.activation(
                    act[:], red[:], AF.Relu,
                    bias=msc_sb[(sf, 'bias')][:, 0:1],
                    scale=msc_sb[(sf, 'scale')][:, 0:1])
                ws = smp.tile([64, cfg.WPB], f32, tag='masws')
                nc.vector.tensor_reduce(
                    out=ws[:],
                    in_=act[:].rearrange("p (w l) -> p w l", l=cfg.LW),
                    axis=mybir.AxisListType.X, op=OP.add)
                if frag is None:
                    frag = ws
                else:
                    frag2 = smp.tile([64, cfg.WPB], f32, tag='masfrag')
                    nc.vector.tensor_add(out=frag2[:], in0=frag[:], in1=ws[:])
                    frag = frag2
            ps_t = psX.tile([cfg.WPB, 64], f32, space="PSUM", tag='aux')
            nc.tensor.transpose(out=ps_t[:], in_=frag[:], identity=id64[:])
            fragT = smp.tile([cfg.WPB, 64], f32, tag='masfragT')
            nc.scalar.activation(fragT[:], ps_t[:], AF.Identity)
            m_ps = psX.tile([P, cfg.GPB], f32, space="PSUM", tag='aux')
            for lb in range(cfg.LB):
                nc.tensor.matmul(
                    m_ps[:], lhsT=wm_sb[:, lb, :],
                    rhs=fragT[:, lb * cfg.GPB:(lb + 1) * cfg.GPB],
                    start=(lb == 0), stop=(lb == cfg.LB - 1))
            m_fm = smp.tile([P, cfg.GPB], f32, tag='masfm')
            nc.scalar.activation(m_fm[:], m_ps[:], AF.Identity,
                                 bias=bm_sb[:, 0:1])
            t_mas = cst.tile([P, cfg.B], f32, tag='tmas')
            nc.vector.tensor_tensor(
                out=t_mas[:].rearrange("p (s g) -> p s g", g=cfg.GPB),
                in0=m_fm[:, None, :].to_broadcast([P, NQ, cfg.GPB]),
                in1=gmask_sb[:, :].rearrange("p (s g) -> p s g", g=cfg.GPB),
                op=OP.mult)

            # bias row + zero row of xwA, then B-half slabs
            zrow = smp.tile([1, cfg.F], fp8, tag='zrow')
            nc.vector.memset(zrow[:], 0.0)
            nc.sync.dma_start(out=xwA[cfg.SH:cfg.SH + 1, :], in_=b8_sb[:])
            nc.sync.dma_start(out=xwA[cfg.SH + 1:cfg.SH + 2, :], in_=zrow[:])

            for sl in range(NSLAB // 2, NSLAB):
                xw_slab(sl)

            # ---------------- gather + scatter + pool
            pool_ps = [psP.tile([cfg.B, 512], f32, space="PSUM",
                                name=f'poolps{fh}') for fh in range(2)]
            blk_ps = {}
            gat_tiles = {}
            # per-group gathers; chunks reference their group's tile
            ch_by_grp = {}
            for ch in chunks:
                ch_by_grp.setdefault(ch['grp'], []).append(ch)

            pooled_n = [0]

            def finish_block(j, ps_pair, phase):
                if phase == 0:
                    # stage A partial (8x scale) into hA as fp8
                    for fh in range(2):
                        nc.scalar.activation(
                            hA[:, j * cfg.F + fh * 512:
                               j * cfg.F + (fh + 1) * 512],
                            ps_pair[fh][:], AF.Identity)
                else:
                    h_t = hp.tile([P, cfg.F], fp8, tag='h')
                    for fh in range(2):
                        nc.scalar.activation(
                            h_t[:, fh * 512:(fh + 1) * 512], ps_pair[fh][:],
                            AF.Lrelu, scale=0.125, alpha=0.01)
                    for fh in range(2):
                        nc.tensor.matmul(
                            pool_ps[fh][:],
                            lhsT=mpool_sb[:, j * cfg.B:(j + 1) * cfg.B],
                            rhs=h_t[:, fh * 512:(fh + 1) * 512],
                            start=(pooled_n[0] == 0),
                            stop=(pooled_n[0] == cfg.NBLK - 1))
                    pooled_n[0] += 1

            for g in range(n_groups):
                hf = groups[g][0]
                src = xwA if hf == 0 else xwB
                gat = gatp.tile([P, GRPU, cfg.F], fp8, tag='gat')
                nc.gpsimd.dma_gather(
                    out_ap=gat[:], in_ap=src[:, :],
                    idxs_ap=idx_sb[:, g * 64:(g + 1) * 64],
                    num_idxs=GRPU * P, num_idxs_reg=GRPU * P,
                    elem_size=cfg.F)
                for ch in ch_by_grp[g]:
                    j, u0 = ch['j'], ch['u0']
                    if ch['first']:
                        pair = [psB.tile([P, 512], f32, space="PSUM",
                                         name=f'blkps{fh}') for fh in range(2)]
                        blk_ps[(j, hf)] = pair
                        if hf == 1:
                            for fh in range(2):
                                nc.tensor.matmul(
                                    pair[fh][:], lhsT=id128[:],
                                    rhs=hA[:, j * cfg.F + fh * 512:
                                           j * cfg.F + (fh + 1) * 512],
                                    start=True, stop=False)
                    pair = blk_ps[(j, hf)]
                    sm0 = (g * GRPU + u0) * P
                    st = ch['first'] and hf == 0
                    sp = ch['last']
                    for fh in range(2):
                        if ch['units'] == 2 and USE_DR:
                            nc.tensor.matmul(
                                pair[fh][:],
                                lhsT=smat_sb[:, sm0:sm0 + 2 * P].rearrange(
                                    "p (i d) -> p i d", i=2),
                                rhs=gat[:, u0:u0 + 2,
                                        fh * 512:(fh + 1) * 512],
                                start=st, stop=sp, perf_mode=DR)
                        else:
                            for i in range(ch['units']):
                                nc.tensor.matmul(
                                    pair[fh][:],
                                    lhsT=smat_sb[:, sm0 + i * P:
                                                 sm0 + (i + 1) * P],
                                    rhs=gat[:, u0 + i,
                                            fh * 512:(fh + 1) * 512],
                                    start=(st and i == 0),
                                    stop=(sp and i == ch['units'] - 1))
                for ch in ch_by_grp[g]:
                    if ch['last']:
                        finish_block(ch['j'], blk_ps.pop((ch['j'], hf)), hf)

            # ---------------- pooled -> x1 partial
            pooled_sb = smp.tile([cfg.B, cfg.F], f32, tag='pooled')
            for fh in range(2):
                nc.scalar.activation(pooled_sb[:, fh * 512:(fh + 1) * 512],
                                     pool_ps[fh][:], AF.Identity,
                                     scale=float(2.0 ** -8))
            pfm = smp.tile([P, 8, cfg.B], f32, tag='pfm')
            for k in range(8):
                tps = psX.tile([P, cfg.B], f32, space="PSUM", tag='aux')
                nc.tensor.transpose(
                    out=tps[:], in_=pooled_sb[:, k * P:(k + 1) * P],
                    identity=id32[:])
                nc.scalar.activation(pfm[:, k, :], tps[:], AF.Identity)
            xps = psX.tile([P, cfg.B], f32, space="PSUM", tag='aux')
            for k in range(8):
                nc.tensor.matmul(xps[:], lhsT=wpf_sb[:, k, :],
                                 rhs=pfm[:, k, :],
                                 start=(k == 0), stop=(k == 7))
            x1p = smp.tile([P, cfg.B], f32, tag='x1p')
            nc.scalar.activation(x1p[:], xps[:], AF.Identity)

            # ---------------- cc packing + allreduce
            t_x1 = smp.tile([P, cfg.B], f32, tag='tx1')
            t_x2 = smp.tile([P, cfg.B], f32, tag='tx2')
            nc.scalar.activation(t_x1[:], x1p[:], AF.Identity,
                                 scale=mask1_sb[:, 0:1])
            nc.scalar.activation(t_x2[:], x1p[:], AF.Identity,
                                 scale=mask2_sb[:, 0:1])
            bounce_in = drp.tile([CC], f32, tag='ccin')
            bounce_out = drp.tile([N_CORES * CC], f32, tag='ccout')
            seg = P * cfg.B
            for i, t in enumerate((t_x1, t_x2, t_mas)):
                nc.sync.dma_start(
                    out=bounce_in[i * seg:(i + 1) * seg].rearrange(
                        "(p f) -> p f", f=cfg.B),
                    in_=t[:])
            nc.gpsimd.collective_compute(
                "AllGather", OP.bypass,
                replica_groups=[list(range(N_CORES))],
                ins=[bounce_in[:].opt()], outs=[bounce_out[:].opt()])
            gath_v = bounce_out[:].rearrange(
                "(r t p f) -> t p r f", r=N_CORES, t=3, p=P)

            def cc_sum(ti, name):
                raw = smp.tile([P, N_CORES, cfg.B], f32, tag='ccraw',
                               name=f'raw{name}')
                nc.sync.dma_start(out=raw[:], in_=gath_v[ti])
                red = smp.tile([P, cfg.B], f32, tag=f'ccred{name}',
                               name=f'red{name}')
                nc.vector.tensor_reduce(
                    out=red[:], in_=raw[:].transpose([0, 2, 1]),
                    axis=mybir.AxisListType.X, op=OP.add)
                return red

            x12 = {}
            for brr, bpf in ((1, bpf1_sb), (2, bpf2_sb)):
                xs = cc_sum(brr - 1, f'x{brr}')
                nc.scalar.activation(xs[:], xs[:], AF.Lrelu,
                                     bias=bpf[:, 0:1], alpha=0.01)
                x12[brr] = xs
            masif_rb = cc_sum(2, 'mas')

            # ---------------- head
            xc1 = {}
            for mh in range(2):
                cps = psX.tile([P, cfg.B], f32, space="PSUM", tag='aux')
                for k2 in range(2):
                    nc.tensor.matmul(
                        cps[:], lhsT=wfc1_sb[:, k2, mh * P:(mh + 1) * P],
                        rhs=x12[k2 + 1][:], start=(k2 == 0), stop=(k2 == 1))
                xcs = smp.tile([P, cfg.B], f32, tag=f'xc{mh}')
                nc.scalar.activation(xcs[:], cps[:], AF.Lrelu,
                                     bias=bfc1_sb[:, mh, 0:1], alpha=0.01)
                xc1[mh] = xcs
            c2ps = psX.tile([64, cfg.B], f32, space="PSUM", tag='aux')
            for k2 in range(2):
                nc.tensor.matmul(c2ps[:], lhsT=wfc2_sb[:, k2, :],
                                 rhs=xc1[k2][:], start=(k2 == 0),
                                 stop=(k2 == 1))
            xc = smp.tile([64, cfg.B], f32, tag='xcf')
            nc.scalar.activation(xc[:], c2ps[:], AF.Lrelu,
                                 bias=bfc2_sb[:, 0:1], alpha=0.01)

            ops = psX.tile([1, cfg.B], f32, space="PSUM", tag='aux')
            nc.tensor.matmul(ops[:], lhsT=wout_x_sb[:], rhs=xc[:],
                             start=True, stop=False)
            nc.tensor.matmul(ops[:], lhsT=wout_m_sb[:], rhs=masif_rb[:],
                             start=False, stop=True)
            res = smp.tile([1, cfg.B], f32, tag='res')
            nc.scalar.activation(res[:], ops[:], AF.Sigmoid,
                                 bias=bout_sb[:, 0:1])
            nc.sync.dma_start(out=out_t[:, :], in_=res[:])

    nc.compile()
    return nc


# ---------------------------------------------------------------- entry
_CACHE = {}


def _run(inputs, cfg, trace=False, tmpdir=None):
    from concourse import bass_utils
    meta, in_maps = _preprocess(inputs, cfg)
    key = tuple((c['hf'], c['j'], c['units'], c['u0'], c['grp'],
                 c['first'], c['last']) for c in meta['chunks'])
    if key not in _CACHE:
        _CACHE.clear()
        _CACHE[key] = _build(cfg, meta)
    nc = _CACHE[key]
    res = bass_utils.run_bass_kernel_spmd(
        nc, in_maps, core_ids=list(range(N_CORES)), trace=trace, tmpdir=tmpdir)
    out = np.asarray(res.results[0]['out'], np.float32).reshape(cfg.B, 1)
    return out, res


def kernel(**inputs) -> np.ndarray:
    cfg = _Cfg()
    out, _ = _run(inputs, cfg)
    return out


# revision 32
# speedup vs baseline: 1.1565x; 1.1104x over previous
"""Trainium2 Bass kernel for nn_GCNN_87668872446200.

Branch-split design over 8 NeuronCores: cores 0-3 run protein branch 1,
cores 4-7 run branch 2.  Within a branch group each core owns a quarter of
the destination nodes and the full F=1024 feature dim.

Per core (fp8 e4m3 data paths, DoubleRow fp8 matmuls):
  - xw' = 8*(x*dinv_row) @ (W*64) / 8   computed on PE in two source-halves,
    written to HBM as two tensors (xwA rows <5120 + bias row, xwB rest)
  - symmetric norm is separated: h = Dinv A Dinv xw + b realized as
    S-matmul with S[e,d] = dinv[d] (bias via a virtual edge to a bias row)
  - dma_gather pulls 1KB fp8 rows per edge; source-half split lets the
    Q7 descriptor emission of half A overlap the xw compute of half B
  - phase A partial sums staged in SBUF (fp8), injected into phase B PSUM
    via an identity matmul; one ACT pass does lrelu(psum/8)
  - per-graph mean-pool as PE matmul (mpool*256 fp8), W_pf applied locally
  - masif branch (8 graphs/core, this core's branch only)
  - one small AllReduce ([3,128,32] f32 = 48KB) + replicated dense head

All 8 cores run ONE identical program; per-core variation is in input data.
"""
import numpy as np

# ---------------------------------------------------------------- constants
N_CORES = 8
P = 128
BLK = 128           # dest nodes per block
NQ = 4              # dest quarters per branch group
GRPU = 8            # 128-idx units per gather call (1024 idxs)

N_NODES, N_EDGES, F_DIM, B_GRAPHS, L_MAS, C_MAS = 10000, 80000, 1024, 32, 800, 16

USE_DR = True       # DoubleRow fp8 matmuls


class _Cfg:
    def __init__(self, n=N_NODES, e=N_EDGES, f=F_DIM, b=B_GRAPHS,
                 l=L_MAS, c=C_MAS):
        self.N, self.E, self.F, self.B, self.L, self.C = n, e, f, b, l, c
        self.NPAD = ((n + 511) // 512) * 512          # 10240
        self.QH = self.NPAD // NQ                     # 2560 dests per core
        self.NBLK = self.QH // BLK                    # 20 blocks
        self.SH = self.NPAD // 2                      # 5120 source-half split
        self.KC2 = f // 256                           # 4 k-pairs
        self.GPB = b // 4                             # 8 graphs per core
        self.LW = l // 80                             # 10
        self.LB = 8                                   # l-blocks
        self.LBS = l // self.LB                       # 100
        self.WPB = self.LBS // self.LW                # 10
        # xwA holds source rows 0..SH-1 plus bias row (SH) and zero row (SH+1)
        self.XWA_ROWS = self.SH + P                   # 5248
        self.XWB_ROWS = self.NPAD                     # full copy (B chunks
                                                      # may gather any source)


def _q8(x):
    import ml_dtypes
    return np.clip(np.asarray(x, np.float32), -240.0, 240.0).astype(
        ml_dtypes.float8_e4m3)


# ---------------------------------------------------------------- host prep
def _edge_plan_core(cfg, edge_index, q):
    """Edges targeting quarter q, deduped per (block, source).
    Returns dict j -> (srcsA, destlistsA, srcsB, destlistsB) where each
    entry is one gather slot (unique source) with its dest list (dests may
    repeat for multi-edges); A entries have src < SH."""
    row = np.asarray(edge_index[0]).astype(np.int64)
    col = np.asarray(edge_index[1]).astype(np.int64)
    loops = np.arange(cfg.N, dtype=np.int64)
    rows = np.concatenate([row, loops])
    cols = np.concatenate([col, loops])
    lo = q * cfg.QH
    sel = (cols >= lo) & (cols < lo + cfg.QH)
    r, c = rows[sel], cols[sel] - lo
    out = {}
    for j in range(cfg.NBLK):
        bsel = (c >= j * BLK) & (c < (j + 1) * BLK)
        rj, cj = r[bsel], c[bsel] - j * BLK
        order = np.argsort(rj, kind='stable')
        rj, cj = rj[order], cj[order]
        ent = {'A': ([], []), 'B': ([], [])}
        i = 0
        while i < len(rj):
            k = i
            while k < len(rj) and rj[k] == rj[i]:
                k += 1
            key = 'A' if rj[i] < cfg.SH else 'B'
            ent[key][0].append(int(rj[i]))
            ent[key][1].append(cj[i:k].copy())
            i = k
        out[j] = (ent['A'][0], ent['A'][1], ent['B'][0], ent['B'][1])
    return out


def _shared_schedule(cfg, plans):
    """Shared chunk schedule (max over the 8 per-core plans).

    Returns chunks: list of dicts with keys
      hf, j, units (1 or 2), u0 (unit offset in group), grp (group index),
      first (starts block), last (ends block's half... block completion is
      tracked at (j,hf==1,last) for B and (j,hf==0,last) for A)
    and n_groups_a / n_groups_b.
    """
    # per-plan slot counts: nA (A-eligible gather slots + 1 bias), ntot.
    # A-overflow can spill to B chunks because xwB holds the full xw, so we
    # search each slot's A-capacity t_a minimizing total units.
    def counts_of(p, j):
        a = len(p[j][0]) + 1                      # + bias slot
        return a, a + len(p[j][2])

    perms = []
    for p in plans:
        tot = [counts_of(p, j)[1] for j in range(cfg.NBLK)]
        perms.append(list(np.argsort(-np.asarray(tot), kind='stable')))

    slot_need = {}
    for s in range(cfg.NBLK):
        cs = [counts_of(p, perm[s]) for p, perm in zip(plans, perms)]
        best = None
        for ta in range(1, GRPU + 3):
            tb = max((tot - min(na, ta * P) + P - 1) // P for na, tot in cs)
            tb = max(tb, 1)
            if best is None or ta + tb <= best[0] + best[1]:
                best = (ta, tb)
        slot_need[(s, 0)], slot_need[(s, 1)] = best

    chunks = []
    groups = []                                   # list of [hf, nunits]
    for hf in range(2):
        space = 0                                 # force new group per half
        for s in range(cfg.NBLK):
            left = slot_need[(s, hf)]
            first = True
            while left:
                if space == 0:
                    groups.append([hf, 0])
                    space = GRPU
                sz = 2 if (left >= 2 and space >= 2) else 1
                chunks.append(dict(hf=hf, j=s, units=sz, u0=GRPU - space,
                                   grp=len(groups) - 1,
                                   first=first, last=(left - sz == 0)))
                space -= sz
                left -= sz
                first = False
                groups[-1][1] = GRPU - space
    return chunks, groups, perms, {k: v for k, v in slot_need.items()}


def _fill_core_gather2(cfg, chunks, groups, plan, dinv, q, perm, slot_need):
    """Per-core idx + smat content for the shared schedule (slot s maps to
    physical block perm[s]; A-entry overflow beyond the slot's A capacity
    spills to the B chunks, which gather from the full xwB)."""
    n_groups = len(groups)
    flat_idx = np.zeros((n_groups, GRPU * P), np.int64)
    smat = np.zeros((n_groups, P, GRPU * P), np.float32)
    dinv8 = _q8(dinv).astype(np.float32)

    # per (slot, hf): entry lists (src-or-None for bias, dest-array)
    entries = {}
    for s in range(cfg.NBLK):
        j = perm[s]
        sA, dA, sB, dB = plan[j]
        capA = slot_need[(s, 0)] * P - 1          # minus bias slot
        ea = [('bias', None)] + [(sA[i], dA[i]) for i in range(min(len(sA),
                                                                  capA))]
        eb = ([(sA[i], dA[i]) for i in range(capA, len(sA))]
              + [(sB[i], dB[i]) for i in range(len(sB))])
        assert len(ea) <= slot_need[(s, 0)] * P
        assert len(eb) <= slot_need[(s, 1)] * P, (s, len(eb))
        entries[(s, 0)] = ea
        entries[(s, 1)] = eb

    consumed = {}
    for ch in chunks:
        s, hf, g, u0 = ch['j'], ch['hf'], ch['grp'], ch['u0']
        j = perm[s]
        ent = entries[(s, hf)]
        off = consumed.get((s, hf), 0)
        cap = ch['units'] * P
        base = u0 * P
        pad_idx = cfg.SH + 1 if hf == 0 else cfg.NPAD - 2
        flat_idx[g, base:base + cap] = pad_idx
        take = min(len(ent) - off, cap)
        for k in range(take):
            src, dst = ent[off + k]
            slot = base + k
            up, pp = slot // P, slot % P
            if src == 'bias':
                flat_idx[g, slot] = cfg.SH
                smat[g, pp, up * P:(up + 1) * P] = 1.0
            else:
                flat_idx[g, slot] = src
                gd = q * cfg.QH + j * BLK + dst
                vals = dinv8[np.minimum(gd, cfg.N - 1)] * (gd < cfg.N)
                np.add.at(smat[g, pp], up * P + dst, vals)
        consumed[(s, hf)] = off + take
    for (s, hf), ent in entries.items():
        assert consumed.get((s, hf), 0) == len(ent), (s, hf)
    return flat_idx, smat


def _wrap_idx_groups(flat_idx):
    """[G, 1024] -> [128, G*64] int16 (16-part wrap, 8x replicated)."""
    g, n = flat_idx.shape
    w = flat_idx.reshape(g, n // 16, 16).transpose(2, 0, 1).reshape(16, -1)
    return np.tile(w, (8, 1)).astype(np.int16)


def _preprocess(inputs, cfg):
    import ml_dtypes
    bf16 = ml_dtypes.bfloat16
    f32 = np.float32

    # --- per-branch shared data
    bdata = {}
    for br in (1, 2):
        x = np.asarray(inputs[f'pro{br}_x'], f32)
        ei = np.asarray(inputs[f'pro{br}_edge_index'])
        batch = np.asarray(inputs[f'pro{br}_batch']).astype(np.int64)
        row = ei[0].astype(np.int64)
        col = ei[1].astype(np.int64)
        deg = np.bincount(np.concatenate([col, np.arange(cfg.N)]),
                          minlength=cfg.N).astype(np.float64)
        dinv = (1.0 / np.sqrt(deg)).astype(f32)
        # xt_dr [NSLAB, 128, KC2*2*512] fp8 of (x*dinv_row)^T, slab-major
        xp = x * dinv[:, None]
        xpT = np.zeros((cfg.F, cfg.NPAD), f32)
        xpT[:, :cfg.N] = xp.T
        nslab = cfg.NPAD // 512
        xt_dr = np.ascontiguousarray(
            xpT.reshape(cfg.KC2, 2, P, nslab, 512).transpose(3, 2, 0, 1, 4)
        ).reshape(nslab, P, cfg.KC2 * 2 * 512)
        # wg_dr [128, KC2*2*F] fp8  (p, c, i, f)
        W = np.asarray(inputs[f'W_g{br}'], f32) * 64.0
        wg_dr = np.ascontiguousarray(
            W.reshape(cfg.KC2, 2, P, cfg.F).transpose(2, 0, 1, 3)
        ).reshape(P, cfg.KC2 * 2 * cfg.F)
        b8 = np.asarray(inputs[f'b_g{br}'], f32) * 8.0
        cnt = np.bincount(batch, minlength=cfg.B).astype(f32)
        plans = [_edge_plan_core(cfg, ei, q) for q in range(NQ)]
        bdata[br] = dict(xt=_q8(xt_dr), wg=_q8(wg_dr), b8=_q8(b8[None, :]),
                         dinv=dinv, batch=batch, cnt=cnt, plans=plans)

    # --- shared chunk schedule (max over all 8 core plans, slot-permuted)
    all_plans = bdata[1]['plans'] + bdata[2]['plans']
    chunks, groups, perms, slot_need = _shared_schedule(cfg, all_plans)
    n_groups = len(groups)
    n_ga = sum(1 for g in groups if g[0] == 0)

    meta = dict(chunks=chunks, groups=groups, n_groups=n_groups, n_ga=n_ga)

    # --- head weights (shared across cores)
    shared = {}
    shared['wfc1'] = np.ascontiguousarray(
        np.asarray(inputs['W_fc1'], f32).reshape(2, P, 256).transpose(1, 0, 2))
    shared['wfc2'] = np.ascontiguousarray(
        np.asarray(inputs['W_fc2'], f32).reshape(2, P, 64).transpose(1, 0, 2))
    shared['bfc1'] = np.ascontiguousarray(
        np.asarray(inputs['b_fc1'], f32).reshape(2, P, 1).transpose(1, 0, 2))
    shared['bfc2'] = np.asarray(inputs['b_fc2'], f32).reshape(64, 1)
    wout = np.asarray(inputs['W_out'], f32)
    shared['wout_x'] = np.ascontiguousarray(wout[0:64])            # [64,1]
    shared['wout_m'] = np.ascontiguousarray(wout[64:192])          # [128,1]
    shared['bout'] = np.asarray(inputs['b_out'], f32).reshape(1, 1)
    shared['bpf1'] = np.asarray(inputs['b_pf1'], f32).reshape(P, 1)
    shared['bpf2'] = np.asarray(inputs['b_pf2'], f32).reshape(P, 1)
    shared['id32'] = np.eye(32, dtype=f32)
    shared['id128_8'] = _q8(np.eye(P, dtype=f32))
    shared['id64'] = np.eye(64, dtype=f32)

    in_maps = []
    for core in range(N_CORES):
        br = 1 + core // NQ
        q = core % NQ
        bd = bdata[br]
        m = dict(shared)
        m['xt'] = bd['xt']
        m['wg'] = bd['wg']
        m['b8row'] = bd['b8']
        # gather plan
        perm = perms[core]
        flat_idx, smat = _fill_core_gather2(
            cfg, chunks, groups, bd['plans'][q], bd['dinv'], q, perm,
            slot_need)
        m['idx'] = _wrap_idx_groups(flat_idx)
        m['smat'] = np.ascontiguousarray(
            smat.transpose(1, 0, 2).reshape(P, n_groups * GRPU * P)).astype(
            ml_dtypes.float8_e4m3)
        # mpool [128, NBLK, B] fp8 (x256); slot j -> physical block perm[j]
        mp = np.zeros((P, cfg.NBLK, cfg.B), f32)
        for j in range(cfg.NBLK):
            nodes = q * cfg.QH + perm[j] * BLK + np.arange(BLK)
            ok = nodes < cfg.N
            gidx = bd['batch'][np.minimum(nodes, cfg.N - 1)]
            val = 256.0 / np.maximum(bd['cnt'][gidx], 1.0) * ok
            mp[np.arange(BLK), j, gidx] = val
        m['mpool'] = _q8(mp.reshape(P, cfg.NBLK * cfg.B))
        # W_pf for this branch  [128, 8*128] f32  (p, k, m)
        wpf = np.asarray(inputs[f'W_pf{br}'], f32)
        m['wpf'] = np.ascontiguousarray(
            wpf.reshape(8, P, P).transpose(1, 0, 2)).reshape(P, 8 * P)
        # branch masks for cc packing
        m['mask1'] = np.full((P, 1), 1.0 if br == 1 else 0.0, f32)
        m['mask2'] = np.full((P, 1), 1.0 if br == 2 else 0.0, f32)
        # masif (this branch only, 8 graphs) laid out [64=(lb,g), C*LBS]
        gs = (core % NQ) * cfg.GPB
        for sfk, name in (('s', 'straight'), ('f', 'flipped')):
            src = np.asarray(inputs[f'mas{br}_{name}'], f32)[gs:gs + cfg.GPB]
            # [g, ch, lb*LBS+l] -> [(lb, g), ch, l]
            r = src.reshape(cfg.GPB, cfg.C, cfg.LB, cfg.LBS).transpose(
                2, 0, 1, 3)
            m[f'mas_{sfk}'] = np.ascontiguousarray(r).reshape(
                64, cfg.C * cfg.LBS)
        # wm128 [10, 8, 128]: this branch's W_m/(2*LW) at rows 64*(br-1)..
        wm = np.zeros((cfg.WPB, cfg.LB, P), f32)
        wsrc = (np.asarray(inputs[f'W_m{br}'], f32) / (2.0 * cfg.LW)).reshape(
            cfg.LB, cfg.WPB, 64)
        wm[:, :, 64 * (br - 1):64 * br] = wsrc.transpose(1, 0, 2)
        m['wm'] = np.ascontiguousarray(wm)
        bm = np.zeros((P, 1), f32)
        bm[64 * (br - 1):64 * br, 0] = np.asarray(inputs[f'b_m{br}'], f32)
        m['bm'] = bm
        gm = np.zeros((P, cfg.B), f32)
        gm[64 * (br - 1):64 * br, gs:gs + cfg.GPB] = 1.0
        m['gmask'] = gm
        for sf, pre in (('s', 'cs'), ('f', 'cf')):
            w = float(np.asarray(inputs[f'{pre}{br}_w'])[0])
            b = float(np.asarray(inputs[f'{pre}{br}_b'])[0])
            m[f'scale_{sf}'] = np.full((64, 1), w / cfg.C, f32)
            m[f'bias_{sf}'] = np.full((64, 1), b, f32)
        in_maps.append(m)
    return meta, in_maps


# ---------------------------------------------------------------- program
def _build(cfg, meta):
    import concourse.bass as bass
    import concourse.bacc as bacc
    import concourse.mybir as mybir
    import concourse.tile as tile

    dt = mybir.dt
    f32 = dt.float32
    fp8 = dt.float8e4
    AF = mybir.ActivationFunctionType
    OP = mybir.AluOpType
    DR = mybir.MatmulPerfMode.DoubleRow if USE_DR else None

    chunks = meta['chunks']
    groups = meta['groups']
    n_groups = meta['n_groups']
    n_ga = meta['n_ga']

    nc = bacc.Bacc("TRN2", target_bir_lowering=False, debug=False,
                   enable_asserts=False, num_devices=N_CORES)

    def din(name, shape, d):
        return nc.dram_tensor(name, list(shape), d, kind="ExternalInput")

    NSLAB = cfg.NPAD // 512
    xt_d = din('xt', (NSLAB, P, cfg.KC2 * 2 * 512), fp8)
    wg_d = din('wg', (P, cfg.KC2 * 2 * cfg.F), fp8)
    b8_d = din('b8row', (1, cfg.F), fp8)
    idx_d = din('idx', (P, n_groups * 64), dt.int16)
    smat_d = din('smat', (P, n_groups * GRPU * P), fp8)
    mpool_d = din('mpool', (P, cfg.NBLK * cfg.B), fp8)
    wpf_d = din('wpf', (P, 8 * P), f32)
    mas_d = {sf: din(f'mas_{sf}', (64, cfg.C * cfg.LBS), f32) for sf in 'sf'}
    wm_d = din('wm', (cfg.WPB, cfg.LB, P), f32)
    bm_d = din('bm', (P, 1), f32)
    gmask_d = din('gmask', (P, cfg.B), f32)
    msc_d = {(sf, kind): din(f'{kind}_{sf}', (64, 1), f32)
             for sf in 'sf' for kind in ('scale', 'bias')}
    wfc1_d = din('wfc1', (P, 2, 256), f32)
    wfc2_d = din('wfc2', (P, 2, 64), f32)
    bfc1_d = din('bfc1', (P, 2, 1), f32)
    bfc2_d = din('bfc2', (64, 1), f32)
    wout_x_d = din('wout_x', (64, 1), f32)
    wout_m_d = din('wout_m', (P, 1), f32)
    bout_d = din('bout', (1, 1), f32)
    bpf1_d = din('bpf1', (P, 1), f32)
    bpf2_d = din('bpf2', (P, 1), f32)
    mask1_d = din('mask1', (P, 1), f32)
    mask2_d = din('mask2', (P, 1), f32)
    id32_d = din('id32', (32, 32), f32)
    id64_d = din('id64', (64, 64), f32)
    id128_d = din('id128_8', (P, P), fp8)

    out_t = nc.dram_tensor('out', [1, cfg.B], f32, kind="ExternalOutput")

    CC = 3 * P * cfg.B          # allreduce payload (f32 elements)

    with tile.TileContext(nc) as tc:
        with tc.tile_pool(name="const", bufs=1) as cst, \
             tc.tile_pool(name="xt", bufs=2) as xtp, \
             tc.tile_pool(name="xwsb", bufs=3) as xwsb, \
             tc.tile_pool(name="gat", bufs=5) as gatp, \
             tc.tile_pool(name="hsb", bufs=2) as hp, \
             tc.tile_pool(name="small", bufs=2) as smp, \
             tc.tile_pool(name="psA", bufs=2, space="PSUM") as psA, \
             tc.tile_pool(name="psBlk", bufs=1, space="PSUM") as psB, \
             tc.tile_pool(name="psPool", bufs=1, space="PSUM") as psP, \
             tc.tile_pool(name="psX", bufs=2, space="PSUM") as psX, \
             tc.tile_pool(name="dram", bufs=1, space="DRAM") as drp:

            def load(pool, src_ap, shape, d, name=None):
                t = pool.tile(list(shape), d, tag=name)
                nc.sync.dma_start(out=t[:], in_=src_ap)
                return t

            # ---------------- xw-critical constants first
            wg_flat = load(cst, wg_d[:, :], (P, cfg.KC2 * 2 * cfg.F), fp8,
                           'wg')
            wg_sb = wg_flat[:].rearrange("p (c i f) -> p c i f", c=cfg.KC2,
                                         i=2)

            # ---------------- xw compute: A half (source rows < SH)
            xwA = drp.tile([cfg.XWA_ROWS, cfg.F], fp8, tag='xwA')
            xwB = drp.tile([cfg.XWB_ROWS, cfg.F], fp8, tag='xwB')

            def xw_slab(sl):
                n0 = sl * 512
                xt_flat = xtp.tile([P, cfg.KC2 * 2 * 512], fp8, tag='xt',
                                   name='xt_t')
                nc.sync.dma_start(out=xt_flat[:], in_=xt_d[sl, :, :])
                xt_t = xt_flat[:].rearrange("p (c i n) -> p c i n", c=cfg.KC2,
                                            i=2)
                for sub in range(4):
                    xw_t = xwsb.tile([P, cfg.F], fp8, tag='xwsb', name='xw_t')
                    for fh in range(2):
                        ps = psA.tile([P, 512], f32, space="PSUM", tag='xwps',
                                      name='xw_ps')
                        for c in range(cfg.KC2):
                            if USE_DR:
                                nc.tensor.matmul(
                                    ps[:],
                                    lhsT=xt_t[:, c, :, sub * P:(sub + 1) * P],
                                    rhs=wg_sb[:, c, :, fh * 512:(fh + 1) * 512],
                                    start=(c == 0), stop=(c == cfg.KC2 - 1),
                                    perf_mode=DR)
                            else:
                                for i in range(2):
                                    nc.tensor.matmul(
                                        ps[:],
                                        lhsT=xt_t[:, c, i,
                                                  sub * P:(sub + 1) * P],
                                        rhs=wg_sb[:, c, i,
                                                  fh * 512:(fh + 1) * 512],
                                        start=(c == 0 and i == 0),
                                        stop=(c == cfg.KC2 - 1 and i == 1))
                        nc.scalar.activation(xw_t[:, fh * 512:(fh + 1) * 512],
                                             ps[:], AF.Identity, scale=0.125)
                    row = n0 + sub * P
                    if row < cfg.SH:
                        nc.sync.dma_start(out=xwA[row:row + P, :], in_=xw_t[:])
                    nc.sync.dma_start(out=xwB[row:row + P, :], in_=xw_t[:])

            for sl in range(NSLAB // 2):
                xw_slab(sl)

            # ---------------- remaining constants (overlap with xw PE)
            idx_sb = load(cst, idx_d[:, :], (P, n_groups * 64), dt.int16, 'idx')
            smat_sb = load(cst, smat_d[:, :], (P, n_groups * GRPU * P), fp8,
                           'smat')
            mpool_sb = load(cst, mpool_d[:, :], (P, cfg.NBLK * cfg.B), fp8,
                            'mpool')
            wpf_flat = load(cst, wpf_d[:, :], (P, 8 * P), f32, 'wpf')
            wpf_sb = wpf_flat[:].rearrange("p (k m) -> p k m", k=8)
            wm_sb = load(cst, wm_d[:, :, :], (cfg.WPB, cfg.LB, P), f32, 'wm')
            bm_sb = load(cst, bm_d[:, :], (P, 1), f32, 'bm')
            gmask_sb = load(cst, gmask_d[:, :], (P, cfg.B), f32, 'gmask')
            msc_sb = {k: load(cst, v[:, :], (64, 1), f32, f'msc{k}')
                      for k, v in msc_d.items()}
            wfc1_sb = load(cst, wfc1_d[:, :, :], (P, 2, 256), f32, 'wfc1')
            wfc2_sb = load(cst, wfc2_d[:, :, :], (P, 2, 64), f32, 'wfc2')
            bfc1_sb = load(cst, bfc1_d[:, :, :], (P, 2, 1), f32, 'bfc1')
            bfc2_sb = load(cst, bfc2_d[:, :], (64, 1), f32, 'bfc2')
            wout_x_sb = load(cst, wout_x_d[:, :], (64, 1), f32, 'woutx')
            wout_m_sb = load(cst, wout_m_d[:, :], (P, 1), f32, 'woutm')
            bout_sb = load(cst, bout_d[:, :], (1, 1), f32, 'bout')
            bpf1_sb = load(cst, bpf1_d[:, :], (P, 1), f32, 'bpf1')
            bpf2_sb = load(cst, bpf2_d[:, :], (P, 1), f32, 'bpf2')
            mask1_sb = load(cst, mask1_d[:, :], (P, 1), f32, 'mask1')
            mask2_sb = load(cst, mask2_d[:, :], (P, 1), f32, 'mask2')
            id32 = load(cst, id32_d[:, :], (32, 32), f32, 'id32')
            id64 = load(cst, id64_d[:, :], (64, 64), f32, 'id64')
            id128 = load(cst, id128_d[:, :], (P, P), fp8, 'id128')
            b8_sb = load(cst, b8_d[:, :], (1, cfg.F), fp8, 'b8')

            hA = cst.tile([P, cfg.NBLK * cfg.F], fp8, tag='hA')

            # ---------------- masif (one branch, 8 graphs -> [128, B] via PE)
            frag = None
            for sf in 'sf':
                tf = smp.tile([64, cfg.C * cfg.LBS], f32, tag='masload',
                              name='mas_t')
                nc.sync.dma_start(out=tf[:], in_=mas_d[sf][:, :])
                t = tf[:].rearrange("p (c l) -> p c l", c=cfg.C)
                red = smp.tile([64, cfg.LBS], f32, tag='masred')
                nc.vector.tensor_reduce(
                    out=red[:], in_=t.transpose([0, 2, 1]),
                    axis=mybir.AxisListType.X, op=OP.add)
                act = smp.tile([64, cfg.LBS], f32, tag='masact')
                nc.scalar.activation(
                    act[:], red[:], AF.Relu,
                    bias=msc_sb[(sf, 'bias')][:, 0:1],
                    scale=msc_sb[(sf, 'scale')][:, 0:1])
                ws = smp.tile([64, cfg.WPB], f32, tag='masws')
                nc.vector.tensor_reduce(
                    out=ws[:],
                    in_=act[:].rearrange("p (w l) -> p w l", l=cfg.LW),
                    axis=mybir.AxisListType.X, op=OP.add)
                if frag is None:
                    frag = ws
                else:
                    frag2 = smp.tile([64, cfg.WPB], f32, tag='masfrag')
                    nc.vector.tensor_add(out=frag2[:], in0=frag[:], in1=ws[:])
                    frag = frag2
            ps_t = psX.tile([cfg.WPB, 64], f32, space="PSUM", tag='aux')
            nc.tensor.transpose(out=ps_t[:], in_=frag[:], identity=id64[:])
            fragT = smp.tile([cfg.WPB, 64], f32, tag='masfragT')
            nc.scalar.activation(fragT[:], ps_t[:], AF.Identity)
            m_ps = psX.tile([P, cfg.GPB], f32, space="PSUM", tag='aux')
            for lb in range(cfg.LB):
                nc.tensor.matmul(
                    m_ps[:], lhsT=wm_sb[:, lb, :],
                    rhs=fragT[:, lb * cfg.GPB:(lb + 1) * cfg.GPB],
                    start=(lb == 0), stop=(lb == cfg.LB - 1))
            m_fm = smp.tile([P, cfg.GPB], f32, tag='masfm')
            nc.scalar.activation(m_fm[:], m_ps[:], AF.Identity,
                                 bias=bm_sb[:, 0:1])
            t_mas = cst.tile([P, cfg.B], f32, tag='tmas')
            nc.vector.tensor_tensor(
                out=t_mas[:].rearrange("p (s g) -> p s g", g=cfg.GPB),
                in0=m_fm[:, None, :].to_broadcast([P, NQ, cfg.GPB]),
                in1=gmask_sb[:, :].rearrange("p (s g) -> p s g", g=cfg.GPB),
                op=OP.mult)

            # bias row + zero row of xwA, then B-half slabs
            zrow = smp.tile([1, cfg.F], fp8, tag='zrow')
            nc.vector.memset(zrow[:], 0.0)
            nc.sync.dma_start(out=xwA[cfg.SH:cfg.SH + 1, :], in_=b8_sb[:])
            nc.sync.dma_start(out=xwA[cfg.SH + 1:cfg.SH + 2, :], in_=zrow[:])

            for sl in range(NSLAB // 2, NSLAB):
                xw_slab(sl)

            # ---------------- gather + scatter + pool
            pool_ps = [psP.tile([cfg.B, 512], f32, space="PSUM",
                                name=f'poolps{fh}') for fh in range(2)]
            blk_ps = {}
            gat_tiles = {}
            # per-group gathers; chunks reference their group's tile
            ch_by_grp = {}
            for ch in chunks:
                ch_by_grp.setdefault(ch['grp'], []).append(ch)

            pooled_n = [0]

            def finish_block(j, ps_pair, phase):
                if phase == 0:
                    # stage A partial (8x scale) into hA as fp8
                    for fh in range(2):
                        nc.scalar.activation(
                            hA[:, j * cfg.F + fh * 512:
                               j * cfg.F + (fh + 1) * 512],
                            ps_pair[fh][:], AF.Identity)
                else:
                    h_t = hp.tile([P, cfg.F], fp8, tag='h')
                    for fh in range(2):
                        nc.scalar.activation(
                            h_t[:, fh * 512:(fh + 1) * 512], ps_pair[fh][:],
                            AF.Lrelu, scale=0.125, alpha=0.01)
                    for fh in range(2):
                        nc.tensor.matmul(
                            pool_ps[fh][:],
                            lhsT=mpool_sb[:, j * cfg.B:(j + 1) * cfg.B],
                            rhs=h_t[:, fh * 512:(fh + 1) * 512],
                            start=(pooled_n[0] == 0),
                            stop=(pooled_n[0] == cfg.NBLK - 1))
                    pooled_n[0] += 1

            for g in range(n_groups):
                hf = groups[g][0]
                src = xwA if hf == 0 else xwB
                gat = gatp.tile([P, GRPU, cfg.F], fp8, tag='gat')
                nc.gpsimd.dma_gather(
                    out_ap=gat[:], in_ap=src[:, :],
                    idxs_ap=idx_sb[:, g * 64:(g + 1) * 64],
                    num_idxs=GRPU * P, num_idxs_reg=GRPU * P,
                    elem_size=cfg.F)
                for ch in ch_by_grp[g]:
                    j, u0 = ch['j'], ch['u0']
                    if ch['first']:
                        pair = [psB.tile([P, 512], f32, space="PSUM",
                                         name=f'blkps{fh}') for fh in range(2)]
                        blk_ps[(j, hf)] = pair
                        if hf == 1:
                            for fh in range(2):
                                nc.tensor.matmul(
                                    pair[fh][:], lhsT=id128[:],
                                    rhs=hA[:, j * cfg.F + fh * 512:
                                           j * cfg.F + (fh + 1) * 512],
                                    start=True, stop=False)
                    pair = blk_ps[(j, hf)]
                    sm0 = (g * GRPU + u0) * P
                    st = ch['first'] and hf == 0
                    sp = ch['last']
                    for fh in range(2):
                        if ch['units'] == 2 and USE_DR:
                            nc.tensor.matmul(
                                pair[fh][:],
                                lhsT=smat_sb[:, sm0:sm0 + 2 * P].rearrange(
                                    "p (i d) -> p i d", i=2),
                                rhs=gat[:, u0:u0 + 2,
                                        fh * 512:(fh + 1) * 512],
                                start=st, stop=sp, perf_mode=DR)
                        else:
                            for i in range(ch['units']):
                                nc.tensor.matmul(
                                    pair[fh][:],
                                    lhsT=smat_sb[:, sm0 + i * P:
                                                 sm0 + (i + 1) * P],
                                    rhs=gat[:, u0 + i,
                                            fh * 512:(fh + 1) * 512],
                                    start=(st and i == 0),
                                    stop=(sp and i == ch['units'] - 1))
                for ch in ch_by_grp[g]:
                    if ch['last']:
                        finish_block(ch['j'], blk_ps.pop((ch['j'], hf)), hf)

            # ---------------- pooled -> x1 partial
            pooled_sb = smp.tile([cfg.B, cfg.F], f32, tag='pooled')
            for fh in range(2):
                nc.scalar.activation(pooled_sb[:, fh * 512:(fh + 1) * 512],
                                     pool_ps[fh][:], AF.Identity,
                                     scale=float(2.0 ** -8))
            pfm = smp.tile([P, 8, cfg.B], f32, tag='pfm')
            for k in range(8):
                tps = psX.tile([P, cfg.B], f32, space="PSUM", tag='aux')
                nc.tensor.transpose(
                    out=tps[:], in_=pooled_sb[:, k * P:(k + 1) * P],
                    identity=id32[:])
                nc.scalar.activation(pfm[:, k, :], tps[:], AF.Identity)
            xps = psX.tile([P, cfg.B], f32, space="PSUM", tag='aux')
            for k in range(8):
                nc.tensor.matmul(xps[:], lhsT=wpf_sb[:, k, :],
                                 rhs=pfm[:, k, :],
                                 start=(k == 0), stop=(k == 7))
            x1p = smp.tile([P, cfg.B], f32, tag='x1p')
            nc.scalar.activation(x1p[:], xps[:], AF.Identity)

            # ---------------- cc packing + allreduce
            t_x1 = smp.tile([P, cfg.B], f32, tag='tx1')
            t_x2 = smp.tile([P, cfg.B], f32, tag='tx2')
            nc.scalar.activation(t_x1[:], x1p[:], AF.Identity,
                                 scale=mask1_sb[:, 0:1])
            nc.scalar.activation(t_x2[:], x1p[:], AF.Identity,
                                 scale=mask2_sb[:, 0:1])
            bounce_in = drp.tile([CC], f32, tag='ccin')
            bounce_out = drp.tile([N_CORES * CC], f32, tag='ccout')
            seg = P * cfg.B
            for i, t in enumerate((t_x1, t_x2, t_mas)):
                nc.sync.dma_start(
                    out=bounce_in[i * seg:(i + 1) * seg].rearrange(
                        "(p f) -> p f", f=cfg.B),
                    in_=t[:])
            nc.gpsimd.collective_compute(
                "AllGather", OP.bypass,
                replica_groups=[list(range(N_CORES))],
                ins=[bounce_in[:].opt()], outs=[bounce_out[:].opt()])
            gath_v = bounce_out[:].rearrange(
                "(r t p f) -> t p r f", r=N_CORES, t=3, p=P)

            def cc_sum(ti, name):
                raw = smp.tile([P, N_CORES, cfg.B], f32, tag='ccraw',
                               name=f'raw{name}')
                nc.sync.dma_start(out=raw[:], in_=gath_v[ti])
                red = smp.tile([P, cfg.B], f32, tag=f'ccred{name}',
                               name=f'red{name}')
                nc.vector.tensor_reduce(
                    out=red[:], in_=raw[:].transpose([0, 2, 1]),
                    axis=mybir.AxisListType.X, op=OP.add)
                return red

            x12 = {}
            for brr, bpf in ((1, bpf1_sb), (2, bpf2_sb)):
                xs = cc_sum(brr - 1, f'x{brr}')
                nc.scalar.activation(xs[:], xs[:], AF.Lrelu,
                                     bias=bpf[:, 0:1], alpha=0.01)
                x12[brr] = xs
            masif_rb = cc_sum(2, 'mas')

            # ---------------- head
            xc1 = {}
            for mh in range(2):
                cps = psX.tile([P, cfg.B], f32, space="PSUM", tag='aux')
                for k2 in range(2):
                    nc.tensor.matmul(
                        cps[:], lhsT=wfc1_sb[:, k2, mh * P:(mh + 1) * P],
                        rhs=x12[k2 + 1][:], start=(k2 == 0), stop=(k2 == 1))
                xcs = smp.tile([P, cfg.B], f32, tag=f'xc{mh}')
                nc.scalar.activation(xcs[:], cps[:], AF.Lrelu,
                                     bias=bfc1_sb[:, mh, 0:1], alpha=0.01)
                xc1[mh] = xcs
            c2ps = psX.tile([64, cfg.B], f32, space="PSUM", tag='aux')
            for k2 in range(2):
                nc.tensor.matmul(c2ps[:], lhsT=wfc2_sb[:, k2, :],
                                 rhs=xc1[k2][:], start=(k2 == 0),
                                 stop=(k2 == 1))
            xc = smp.tile([64, cfg.B], f32, tag='xcf')
            nc.scalar.activation(xc[:], c2ps[:], AF.Lrelu,
                                 bias=bfc2_sb[:, 0:1], alpha=0.01)

            ops = psX.tile([1, cfg.B], f32, space="PSUM", tag='aux')
            nc.tensor.matmul(ops[:], lhsT=wout_x_sb[:], rhs=xc[:],
                             start=True, stop=False)
            nc.tensor.matmul(ops[:], lhsT=wout_m_sb[:], rhs=masif_rb[:],
                             start=False, stop=True)
            res = smp.tile([1, cfg.B], f32, tag='res')
            nc.scalar.activation(res[:], ops[:], AF.Sigmoid,
                                 bias=bout_sb[:, 0:1])
            nc.sync.dma_start(out=out_t[:, :], in_=res[:])

    nc.compile()
    return nc


# ---------------------------------------------------------------- entry
_CACHE = {}


def _run(inputs, cfg, trace=False, tmpdir=None):
    from concourse import bass_utils
    meta, in_maps = _preprocess(inputs, cfg)
    key = tuple((c['hf'], c['j'], c['units'], c['u0'], c['grp'],
                 c['first'], c['last']) for c in meta['chunks'])
    if key not in _CACHE:
        _CACHE.clear()
        _CACHE[key] = _build(cfg, meta)
    nc = _CACHE[key]
    res = bass_utils.run_bass_kernel_spmd(
        nc, in_maps, core_ids=list(range(N_CORES)), trace=trace, tmpdir=tmpdir)
    out = np.asarray(res.results[0]['out'], np.float32).reshape(cfg.B, 1)
    return out, res


def kernel(**inputs) -> np.ndarray:
    cfg = _Cfg()
    out, _ = _run(inputs, cfg)
    return out


# revision 40
# speedup vs baseline: 1.1645x; 1.0069x over previous
"""Trainium2 Bass kernel for nn_GCNN_87668872446200.

Branch-split design over 8 NeuronCores: cores 0-3 run protein branch 1,
cores 4-7 run branch 2.  Within a branch group each core owns a quarter of
the destination nodes and the full F=1024 feature dim.

Per core (fp8 e4m3 data paths, DoubleRow fp8 matmuls):
  - xw' = 8*(x*dinv_row) @ (W*64) / 8   computed on PE in two source-halves,
    written to HBM as two tensors (xwA rows <5120 + bias row, xwB rest)
  - symmetric norm is separated: h = Dinv A Dinv xw + b realized as
    S-matmul with S[e,d] = dinv[d] (bias via a virtual edge to a bias row)
  - dma_gather pulls 1KB fp8 rows per edge; source-half split lets the
    Q7 descriptor emission of half A overlap the xw compute of half B
  - phase A partial sums staged in SBUF (fp8), injected into phase B PSUM
    via an identity matmul; one ACT pass does lrelu(psum/8)
  - per-graph mean-pool as PE matmul (mpool*256 fp8), W_pf applied locally
  - masif branch (8 graphs/core, this core's branch only)
  - one small AllReduce ([3,128,32] f32 = 48KB) + replicated dense head

All 8 cores run ONE identical program; per-core variation is in input data.
"""
import numpy as np

# ---------------------------------------------------------------- constants
N_CORES = 8
P = 128
BLK = 128           # dest nodes per block
NQ = 4              # dest quarters per branch group
GRPU = 8            # 128-idx units per gather call (1024 idxs)

N_NODES, N_EDGES, F_DIM, B_GRAPHS, L_MAS, C_MAS = 10000, 80000, 1024, 32, 800, 16

USE_DR = True       # DoubleRow fp8 matmuls


class _Cfg:
    def __init__(self, n=N_NODES, e=N_EDGES, f=F_DIM, b=B_GRAPHS,
                 l=L_MAS, c=C_MAS):
        self.N, self.E, self.F, self.B, self.L, self.C = n, e, f, b, l, c
        self.NPAD = ((n + 511) // 512) * 512          # 10240
        self.QH = self.NPAD // NQ                     # 2560 dests per core
        self.NBLK = self.QH // BLK                    # 20 blocks
        self.SH = self.NPAD // 2                      # 5120 source A/B split
        self.KC2 = f // 256                           # 4 k-pairs
        self.GPB = b // 4                             # 8 graphs per core
        self.LW = l // 80                             # 10
        self.LB = 8                                   # l-blocks
        self.LBS = l // self.LB                       # 100
        self.WPB = self.LBS // self.LW                # 10
        # xwA holds source rows 0..SH-1 plus bias row (SH) and zero row (SH+1)
        self.XWA_ROWS = self.SH + P                   # 5248
        self.XWB_ROWS = self.NPAD                     # full copy (B chunks
                                                      # may gather any source)


def _q8(x):
    import ml_dtypes
    return np.clip(np.asarray(x, np.float32), -240.0, 240.0).astype(
        ml_dtypes.float8_e4m3)


# ---------------------------------------------------------------- host prep
def _edge_plan_core(cfg, edge_index, q):
    """Edges targeting quarter q, deduped per (block, source).
    Returns dict j -> (srcsA, destlistsA, srcsB, destlistsB) where each
    entry is one gather slot (unique source) with its dest list (dests may
    repeat for multi-edges); A entries have src < SH."""
    row = np.asarray(edge_index[0]).astype(np.int64)
    col = np.asarray(edge_index[1]).astype(np.int64)
    loops = np.arange(cfg.N, dtype=np.int64)
    rows = np.concatenate([row, loops])
    cols = np.concatenate([col, loops])
    lo = q * cfg.QH
    sel = (cols >= lo) & (cols < lo + cfg.QH)
    r, c = rows[sel], cols[sel] - lo
    out = {}
    for j in range(cfg.NBLK):
        bsel = (c >= j * BLK) & (c < (j + 1) * BLK)
        rj, cj = r[bsel], c[bsel] - j * BLK
        order = np.argsort(rj, kind='stable')
        rj, cj = rj[order], cj[order]
        ent = {'A': ([], []), 'B': ([], [])}
        i = 0
        while i < len(rj):
            k = i
            while k < len(rj) and rj[k] == rj[i]:
                k += 1
            key = 'A' if rj[i] < cfg.SH else 'B'
            ent[key][0].append(int(rj[i]))
            ent[key][1].append(cj[i:k].copy())
            i = k
        out[j] = (ent['A'][0], ent['A'][1], ent['B'][0], ent['B'][1])
    return out


def _shared_schedule(cfg, plans):
    """Shared chunk schedule (max over the 8 per-core plans).

    Returns chunks: list of dicts with keys
      hf, j, units (1 or 2), u0 (unit offset in group), grp (group index),
      first (starts block), last (ends block's half... block completion is
      tracked at (j,hf==1,last) for B and (j,hf==0,last) for A)
    and n_groups_a / n_groups_b.
    """
    # per-plan slot counts: nA (A-eligible gather slots + 1 bias), ntot.
    # A-overflow can spill to B chunks because xwB holds the full xw, so we
    # search each slot's A-capacity t_a minimizing total units.
    def counts_of(p, j):
        a = len(p[j][0]) + 1                      # + bias slot
        return a, a + len(p[j][2])

    perms = []
    for p in plans:
        tot = [counts_of(p, j)[1] for j in range(cfg.NBLK)]
        perms.append(list(np.argsort(-np.asarray(tot), kind='stable')))

    slot_need = {}
    for s in range(cfg.NBLK):
        cs = [counts_of(p, perm[s]) for p, perm in zip(plans, perms)]
        best = None
        for ta in range(1, GRPU + 3):
            tb = max((tot - min(na, ta * P) + P - 1) // P for na, tot in cs)
            tb = max(tb, 1)
            if best is None or ta + tb <= best[0] + best[1]:
                best = (ta, tb)
        slot_need[(s, 0)], slot_need[(s, 1)] = best

    chunks = []
    groups = []                                   # list of [hf, nunits]
    for hf in range(2):
        space = 0                                 # force new group per half
        for s in range(cfg.NBLK):
            left = slot_need[(s, hf)]
            first = True
            while left:
                if space == 0:
                    groups.append([hf, 0])
                    space = GRPU
                sz = 2 if (left >= 2 and space >= 2) else 1
                chunks.append(dict(hf=hf, j=s, units=sz, u0=GRPU - space,
                                   grp=len(groups) - 1,
                                   first=first, last=(left - sz == 0)))
                space -= sz
                left -= sz
                first = False
                groups[-1][1] = GRPU - space
    return chunks, groups, perms, {k: v for k, v in slot_need.items()}


def _fill_core_gather2(cfg, chunks, groups, plan, dinv, q, perm, slot_need):
    """Per-core idx + smat content for the shared schedule (slot s maps to
    physical block perm[s]; A-entry overflow beyond the slot's A capacity
    spills to the B chunks, which gather from the full xwB)."""
    n_groups = len(groups)
    flat_idx = np.zeros((n_groups, GRPU * P), np.int64)
    smat = np.zeros((n_groups, P, GRPU * P), np.float32)
    dinv8 = _q8(dinv).astype(np.float32)

    # per (slot, hf): entry lists (src-or-None for bias, dest-array)
    entries = {}
    for s in range(cfg.NBLK):
        j = perm[s]
        sA, dA, sB, dB = plan[j]
        capA = slot_need[(s, 0)] * P - 1          # minus bias slot
        ea = [('bias', None)] + [(sA[i], dA[i]) for i in range(min(len(sA),
                                                                  capA))]
        eb = ([(sA[i], dA[i]) for i in range(capA, len(sA))]
              + [(sB[i], dB[i]) for i in range(len(sB))])
        assert len(ea) <= slot_need[(s, 0)] * P
        assert len(eb) <= slot_need[(s, 1)] * P, (s, len(eb))
        entries[(s, 0)] = ea
        entries[(s, 1)] = eb

    consumed = {}
    for ch in chunks:
        s, hf, g, u0 = ch['j'], ch['hf'], ch['grp'], ch['u0']
        j = perm[s]
        ent = entries[(s, hf)]
        off = consumed.get((s, hf), 0)
        cap = ch['units'] * P
        base = u0 * P
        pad_idx = cfg.SH + 1 if hf == 0 else cfg.NPAD - 2
        flat_idx[g, base:base + cap] = pad_idx
        take = min(len(ent) - off, cap)
        for k in range(take):
            src, dst = ent[off + k]
            slot = base + k
            up, pp = slot // P, slot % P
            if src == 'bias':
                flat_idx[g, slot] = cfg.SH
                smat[g, pp, up * P:(up + 1) * P] = 1.0
            else:
                flat_idx[g, slot] = src
                gd = q * cfg.QH + j * BLK + dst
                vals = dinv8[np.minimum(gd, cfg.N - 1)] * (gd < cfg.N)
                np.add.at(smat[g, pp], up * P + dst, vals)
        consumed[(s, hf)] = off + take
    for (s, hf), ent in entries.items():
        assert consumed.get((s, hf), 0) == len(ent), (s, hf)
    return flat_idx, smat


def _wrap_idx_groups(flat_idx):
    """[G, 1024] -> [128, G*64] int16 (16-part wrap, 8x replicated)."""
    g, n = flat_idx.shape
    w = flat_idx.reshape(g, n // 16, 16).transpose(2, 0, 1).reshape(16, -1)
    return np.tile(w, (8, 1)).astype(np.int16)


def _preprocess(inputs, cfg):
    import ml_dtypes
    bf16 = ml_dtypes.bfloat16
    f32 = np.float32

    # --- per-branch shared data
    bdata = {}
    for br in (1, 2):
        x = np.asarray(inputs[f'pro{br}_x'], f32)
        ei = np.asarray(inputs[f'pro{br}_edge_index'])
        batch = np.asarray(inputs[f'pro{br}_batch']).astype(np.int64)
        row = ei[0].astype(np.int64)
        col = ei[1].astype(np.int64)
        deg = np.bincount(np.concatenate([col, np.arange(cfg.N)]),
                          minlength=cfg.N).astype(np.float64)
        dinv = (1.0 / np.sqrt(deg)).astype(f32)
        # xt_dr [NSLAB, 128, KC2*2*512] fp8 of (x*dinv_row)^T, slab-major
        xp = x * dinv[:, None]
        xpT = np.zeros((cfg.F, cfg.NPAD), f32)
        xpT[:, :cfg.N] = xp.T
        nslab = cfg.NPAD // 512
        xt_dr = np.ascontiguousarray(
            xpT.reshape(cfg.KC2, 2, P, nslab, 512).transpose(3, 2, 0, 1, 4)
        ).reshape(nslab, P, cfg.KC2 * 2 * 512)
        # wg_dr [128, KC2*2*F] fp8  (p, c, i, f)
        W = np.asarray(inputs[f'W_g{br}'], f32) * 64.0
        wg_dr = np.ascontiguousarray(
            W.reshape(cfg.KC2, 2, P, cfg.F).transpose(2, 0, 1, 3)
        ).reshape(P, cfg.KC2 * 2 * cfg.F)
        b8 = np.asarray(inputs[f'b_g{br}'], f32) * 8.0
        cnt = np.bincount(batch, minlength=cfg.B).astype(f32)
        plans = [_edge_plan_core(cfg, ei, q) for q in range(NQ)]
        bdata[br] = dict(xt=_q8(xt_dr), wg=_q8(wg_dr), b8=_q8(b8[None, :]),
                         dinv=dinv, batch=batch, cnt=cnt, plans=plans)

    # --- shared chunk schedule (max over all 8 core plans, slot-permuted)
    all_plans = bdata[1]['plans'] + bdata[2]['plans']
    chunks, groups, perms, slot_need = _shared_schedule(cfg, all_plans)
    n_groups = len(groups)
    n_ga = sum(1 for g in groups if g[0] == 0)

    meta = dict(chunks=chunks, groups=groups, n_groups=n_groups, n_ga=n_ga)

    # --- head weights (shared across cores)
    shared = {}
    shared['wfc1'] = np.ascontiguousarray(
        np.asarray(inputs['W_fc1'], f32).reshape(2, P, 256).transpose(1, 0, 2))
    shared['wfc2'] = np.ascontiguousarray(
        np.asarray(inputs['W_fc2'], f32).reshape(2, P, 64).transpose(1, 0, 2))
    shared['bfc1'] = np.ascontiguousarray(
        np.asarray(inputs['b_fc1'], f32).reshape(2, P, 1).transpose(1, 0, 2))
    shared['bfc2'] = np.asarray(inputs['b_fc2'], f32).reshape(64, 1)
    wout = np.asarray(inputs['W_out'], f32)
    shared['wout_x'] = np.ascontiguousarray(wout[0:64])            # [64,1]
    shared['wout_m'] = np.ascontiguousarray(wout[64:192])          # [128,1]
    shared['bout'] = np.asarray(inputs['b_out'], f32).reshape(1, 1)
    shared['bpf1'] = np.asarray(inputs['b_pf1'], f32).reshape(P, 1)
    shared['bpf2'] = np.asarray(inputs['b_pf2'], f32).reshape(P, 1)
    shared['id32'] = np.eye(32, dtype=f32)
    shared['id128_8'] = _q8(np.eye(P, dtype=f32))
    shared['id64'] = np.eye(64, dtype=f32)

    in_maps = []
    for core in range(N_CORES):
        br = 1 + core // NQ
        q = core % NQ
        bd = bdata[br]
        m = dict(shared)
        m['xt'] = bd['xt']
        m['wg'] = bd['wg']
        m['b8row'] = bd['b8']
        # gather plan
        perm = perms[core]
        flat_idx, smat = _fill_core_gather2(
            cfg, chunks, groups, bd['plans'][q], bd['dinv'], q, perm,
            slot_need)
        m['idx'] = _wrap_idx_groups(flat_idx)
        m['smat'] = np.ascontiguousarray(
            smat.transpose(1, 0, 2).reshape(P, n_groups * GRPU * P)).astype(
            ml_dtypes.float8_e4m3)
        # mpool [128, NBLK, B] fp8 (x256); slot j -> physical block perm[j]
        mp = np.zeros((P, cfg.NBLK, cfg.B), f32)
        for j in range(cfg.NBLK):
            nodes = q * cfg.QH + perm[j] * BLK + np.arange(BLK)
            ok = nodes < cfg.N
            gidx = bd['batch'][np.minimum(nodes, cfg.N - 1)]
            val = 256.0 / np.maximum(bd['cnt'][gidx], 1.0) * ok
            mp[np.arange(BLK), j, gidx] = val
        m['mpool'] = _q8(mp.reshape(P, cfg.NBLK * cfg.B))
        # W_pf for this branch  [128, 8*128] f32  (p, k, m)
        wpf = np.asarray(inputs[f'W_pf{br}'], f32)
        m['wpf'] = np.ascontiguousarray(
            wpf.reshape(8, P, P).transpose(1, 0, 2)).reshape(P, 8 * P)
        # branch masks for cc packing
        m['mask1'] = np.full((P, 1), 1.0 if br == 1 else 0.0, f32)
        m['mask2'] = np.full((P, 1), 1.0 if br == 2 else 0.0, f32)
        # masif (this branch only, 8 graphs) laid out [64=(lb,g), C*LBS]
        gs = (core % NQ) * cfg.GPB
        for sfk, name in (('s', 'straight'), ('f', 'flipped')):
            src = np.asarray(inputs[f'mas{br}_{name}'], f32)[gs:gs + cfg.GPB]
            # [g, ch, lb*LBS+l] -> [(lb, g), ch, l]
            r = src.reshape(cfg.GPB, cfg.C, cfg.LB, cfg.LBS).transpose(
                2, 0, 1, 3)
            m[f'mas_{sfk}'] = np.ascontiguousarray(r).reshape(
                64, cfg.C * cfg.LBS)
        # wm128 [10, 8, 128]: this branch's W_m/(2*LW) at rows 64*(br-1)..
        wm = np.zeros((cfg.WPB, cfg.LB, P), f32)
        wsrc = (np.asarray(inputs[f'W_m{br}'], f32) / (2.0 * cfg.LW)).reshape(
            cfg.LB, cfg.WPB, 64)
        wm[:, :, 64 * (br - 1):64 * br] = wsrc.transpose(1, 0, 2)
        m['wm'] = np.ascontiguousarray(wm)
        bm = np.zeros((P, 1), f32)
        bm[64 * (br - 1):64 * br, 0] = np.asarray(inputs[f'b_m{br}'], f32)
        m['bm'] = bm
        gm = np.zeros((P, cfg.B), f32)
        gm[64 * (br - 1):64 * br, gs:gs + cfg.GPB] = 1.0
        m['gmask'] = gm
        for sf, pre in (('s', 'cs'), ('f', 'cf')):
            w = float(np.asarray(inputs[f'{pre}{br}_w'])[0])
            b = float(np.asarray(inputs[f'{pre}{br}_b'])[0])
            m[f'scale_{sf}'] = np.full((64, 1), w / cfg.C, f32)
            m[f'bias_{sf}'] = np.full((64, 1), b, f32)
        in_maps.append(m)
    return meta, in_maps


# ---------------------------------------------------------------- program
def _build(cfg, meta):
    import concourse.bass as bass
    import concourse.bacc as bacc
    import concourse.mybir as mybir
    import concourse.tile as tile

    dt = mybir.dt
    f32 = dt.float32
    fp8 = dt.float8e4
    AF = mybir.ActivationFunctionType
    OP = mybir.AluOpType
    DR = mybir.MatmulPerfMode.DoubleRow if USE_DR else None

    chunks = meta['chunks']
    groups = meta['groups']
    n_groups = meta['n_groups']
    n_ga = meta['n_ga']

    nc = bacc.Bacc("TRN2", target_bir_lowering=False, debug=False,
                   enable_asserts=False, num_devices=N_CORES)

    def din(name, shape, d):
        return nc.dram_tensor(name, list(shape), d, kind="ExternalInput")

    NSLAB = cfg.NPAD // 512
    xt_d = din('xt', (NSLAB, P, cfg.KC2 * 2 * 512), fp8)
    wg_d = din('wg', (P, cfg.KC2 * 2 * cfg.F), fp8)
    b8_d = din('b8row', (1, cfg.F), fp8)
    idx_d = din('idx', (P, n_groups * 64), dt.int16)
    smat_d = din('smat', (P, n_groups * GRPU * P), fp8)
    mpool_d = din('mpool', (P, cfg.NBLK * cfg.B), fp8)
    wpf_d = din('wpf', (P, 8 * P), f32)
    mas_d = {sf: din(f'mas_{sf}', (64, cfg.C * cfg.LBS), f32) for sf in 'sf'}
    wm_d = din('wm', (cfg.WPB, cfg.LB, P), f32)
    bm_d = din('bm', (P, 1), f32)
    gmask_d = din('gmask', (P, cfg.B), f32)
    msc_d = {(sf, kind): din(f'{kind}_{sf}', (64, 1), f32)
             for sf in 'sf' for kind in ('scale', 'bias')}
    wfc1_d = din('wfc1', (P, 2, 256), f32)
    wfc2_d = din('wfc2', (P, 2, 64), f32)
    bfc1_d = din('bfc1', (P, 2, 1), f32)
    bfc2_d = din('bfc2', (64, 1), f32)
    wout_x_d = din('wout_x', (64, 1), f32)
    wout_m_d = din('wout_m', (P, 1), f32)
    bout_d = din('bout', (1, 1), f32)
    bpf1_d = din('bpf1', (P, 1), f32)
    bpf2_d = din('bpf2', (P, 1), f32)
    mask1_d = din('mask1', (P, 1), f32)
    mask2_d = din('mask2', (P, 1), f32)
    id32_d = din('id32', (32, 32), f32)
    id64_d = din('id64', (64, 64), f32)
    id128_d = din('id128_8', (P, P), fp8)

    out_t = nc.dram_tensor('out', [1, cfg.B], f32, kind="ExternalOutput")

    CC = 3 * P * cfg.B          # allreduce payload (f32 elements)

    with tile.TileContext(nc) as tc:
        with tc.tile_pool(name="const", bufs=1) as cst, \
             tc.tile_pool(name="xt", bufs=2) as xtp, \
             tc.tile_pool(name="xwsb", bufs=3) as xwsb, \
             tc.tile_pool(name="gat", bufs=5) as gatp, \
             tc.tile_pool(name="hsb", bufs=2) as hp, \
             tc.tile_pool(name="small", bufs=2) as smp, \
             tc.tile_pool(name="psA", bufs=2, space="PSUM") as psA, \
             tc.tile_pool(name="psBlk", bufs=1, space="PSUM") as psB, \
             tc.tile_pool(name="psPool", bufs=1, space="PSUM") as psP, \
             tc.tile_pool(name="psX", bufs=2, space="PSUM") as psX, \
             tc.tile_pool(name="dram", bufs=1, space="DRAM") as drp:

            def load(pool, src_ap, shape, d, name=None):
                t = pool.tile(list(shape), d, tag=name)
                nc.sync.dma_start(out=t[:], in_=src_ap)
                return t

            # ---------------- xw-critical constants first
            wg_flat = load(cst, wg_d[:, :], (P, cfg.KC2 * 2 * cfg.F), fp8,
                           'wg')
            wg_sb = wg_flat[:].rearrange("p (c i f) -> p c i f", c=cfg.KC2,
                                         i=2)

            # ---------------- xw compute: A half (source rows < SH)
            xwA = drp.tile([cfg.XWA_ROWS, cfg.F], fp8, tag='xwA')
            xwB = drp.tile([cfg.XWB_ROWS, cfg.F], fp8, tag='xwB')

            def xw_slab(sl):
                n0 = sl * 512
                xt_flat = xtp.tile([P, cfg.KC2 * 2 * 512], fp8, tag='xt',
                                   name='xt_t')
                nc.sync.dma_start(out=xt_flat[:], in_=xt_d[sl, :, :])
                xt_t = xt_flat[:].rearrange("p (c i n) -> p c i n", c=cfg.KC2,
                                            i=2)
                for sub in range(4):
                    xw_t = xwsb.tile([P, cfg.F], fp8, tag='xwsb', name='xw_t')
                    for fh in range(2):
                        ps = psA.tile([P, 512], f32, space="PSUM", tag='xwps',
                                      name='xw_ps')
                        for c in range(cfg.KC2):
                            if USE_DR:
                                nc.tensor.matmul(
                                    ps[:],
                                    lhsT=xt_t[:, c, :, sub * P:(sub + 1) * P],
                                    rhs=wg_sb[:, c, :, fh * 512:(fh + 1) * 512],
                                    start=(c == 0), stop=(c == cfg.KC2 - 1),
                                    perf_mode=DR)
                            else:
                                for i in range(2):
                                    nc.tensor.matmul(
                                        ps[:],
                                        lhsT=xt_t[:, c, i,
                                                  sub * P:(sub + 1) * P],
                                        rhs=wg_sb[:, c, i,
                                                  fh * 512:(fh + 1) * 512],
                                        start=(c == 0 and i == 0),
                                        stop=(c == cfg.KC2 - 1 and i == 1))
                        nc.scalar.activation(xw_t[:, fh * 512:(fh + 1) * 512],
                                             ps[:], AF.Identity, scale=0.125)
                    row = n0 + sub * P
                    if row < cfg.SH:
                        nc.sync.dma_start(out=xwA[row:row + P, :], in_=xw_t[:])
                    nc.sync.dma_start(out=xwB[row:row + P, :], in_=xw_t[:])

            ASLAB = cfg.SH // 512
            for sl in range(ASLAB):
                xw_slab(sl)

            # ---------------- remaining constants (overlap with xw PE)
            idx_sb = load(cst, idx_d[:, :], (P, n_groups * 64), dt.int16, 'idx')
            smat_sb = load(cst, smat_d[:, :], (P, n_groups * GRPU * P), fp8,
                           'smat')
            mpool_sb = load(cst, mpool_d[:, :], (P, cfg.NBLK * cfg.B), fp8,
                            'mpool')
            wpf_flat = load(cst, wpf_d[:, :], (P, 8 * P), f32, 'wpf')
            wpf_sb = wpf_flat[:].rearrange("p (k m) -> p k m", k=8)
            wm_sb = load(cst, wm_d[:, :, :], (cfg.WPB, cfg.LB, P), f32, 'wm')
            bm_sb = load(cst, bm_d[:, :], (P, 1), f32, 'bm')
            gmask_sb = load(cst, gmask_d[:, :], (P, cfg.B), f32, 'gmask')
            msc_sb = {k: load(cst, v[:, :], (64, 1), f32, f'msc{k}')
                      for k, v in msc_d.items()}
            wfc1_sb = load(cst, wfc1_d[:, :, :], (P, 2, 256), f32, 'wfc1')
            wfc2_sb = load(cst, wfc2_d[:, :, :], (P, 2, 64), f32, 'wfc2')
            bfc1_sb = load(cst, bfc1_d[:, :, :], (P, 2, 1), f32, 'bfc1')
            bfc2_sb = load(cst, bfc2_d[:, :], (64, 1), f32, 'bfc2')
            wout_x_sb = load(cst, wout_x_d[:, :], (64, 1), f32, 'woutx')
            wout_m_sb = load(cst, wout_m_d[:, :], (P, 1), f32, 'woutm')
            bout_sb = load(cst, bout_d[:, :], (1, 1), f32, 'bout')
            bpf1_sb = load(cst, bpf1_d[:, :], (P, 1), f32, 'bpf1')
            bpf2_sb = load(cst, bpf2_d[:, :], (P, 1), f32, 'bpf2')
            mask1_sb = load(cst, mask1_d[:, :], (P, 1), f32, 'mask1')
            mask2_sb = load(cst, mask2_d[:, :], (P, 1), f32, 'mask2')
            id32 = load(cst, id32_d[:, :], (32, 32), f32, 'id32')
            id64 = load(cst, id64_d[:, :], (64, 64), f32, 'id64')
            id128 = load(cst, id128_d[:, :], (P, P), fp8, 'id128')
            b8_sb = load(cst, b8_d[:, :], (1, cfg.F), fp8, 'b8')

            hA = cst.tile([P, cfg.NBLK * cfg.F], fp8, tag='hA')

            # ---------------- masif (one branch, 8 graphs -> [128, B] via PE)
            frag = None
            for sf in 'sf':
                tf = smp.tile([64, cfg.C * cfg.LBS], f32, tag='masload',
                              name='mas_t')
                nc.sync.dma_start(out=tf[:], in_=mas_d[sf][:, :])
                t = tf[:].rearrange("p (c l) -> p c l", c=cfg.C)
                red = smp.tile([64, cfg.LBS], f32, tag='masred')
                nc.vector.tensor_reduce(
                    out=red[:], in_=t.transpose([0, 2, 1]),
                    axis=mybir.AxisListType.X, op=OP.add)
                act = smp.tile([64, cfg.LBS], f32, tag='masact')
                nc.scalar.activation(
                    act[:], red[:], AF.Relu,
                    bias=msc_sb[(sf, 'bias')][:, 0:1],
                    scale=msc_sb[(sf, 'scale')][:, 0:1])
                ws = smp.tile([64, cfg.WPB], f32, tag='masws')
                nc.vector.tensor_reduce(
                    out=ws[:],
                    in_=act[:].rearrange("p (w l) -> p w l", l=cfg.LW),
                    axis=mybir.AxisListType.X, op=OP.add)
                if frag is None:
                    frag = ws
                else:
                    frag2 = smp.tile([64, cfg.WPB], f32, tag='masfrag')
                    nc.vector.tensor_add(out=frag2[:], in0=frag[:], in1=ws[:])
                    frag = frag2
            ps_t = psX.tile([cfg.WPB, 64], f32, space="PSUM", tag='aux')
            nc.tensor.transpose(out=ps_t[:], in_=frag[:], identity=id64[:])
            fragT = smp.tile([cfg.WPB, 64], f32, tag='masfragT')
            nc.scalar.activation(fragT[:], ps_t[:], AF.Identity)
            m_ps = psX.tile([P, cfg.GPB], f32, space="PSUM", tag='aux')
            for lb in range(cfg.LB):
                nc.tensor.matmul(
                    m_ps[:], lhsT=wm_sb[:, lb, :],
                    rhs=fragT[:, lb * cfg.GPB:(lb + 1) * cfg.GPB],
                    start=(lb == 0), stop=(lb == cfg.LB - 1))
            m_fm = smp.tile([P, cfg.GPB], f32, tag='masfm')
            nc.scalar.activation(m_fm[:], m_ps[:], AF.Identity,
                                 bias=bm_sb[:, 0:1])
            t_mas = cst.tile([P, cfg.B], f32, tag='tmas')
            nc.vector.tensor_tensor(
                out=t_mas[:].rearrange("p (s g) -> p s g", g=cfg.GPB),
                in0=m_fm[:, None, :].to_broadcast([P, NQ, cfg.GPB]),
                in1=gmask_sb[:, :].rearrange("p (s g) -> p s g", g=cfg.GPB),
                op=OP.mult)

            # bias row + zero row of xwA, then B-half slabs
            zrow = smp.tile([1, cfg.F], fp8, tag='zrow')
            nc.vector.memset(zrow[:], 0.0)
            nc.sync.dma_start(out=xwA[cfg.SH:cfg.SH + 1, :], in_=b8_sb[:])
            nc.sync.dma_start(out=xwA[cfg.SH + 1:cfg.SH + 2, :], in_=zrow[:])

            for sl in range(ASLAB, NSLAB):
                xw_slab(sl)

            # ---------------- gather + scatter + pool
            pool_ps = [psP.tile([cfg.B, 512], f32, space="PSUM",
                                name=f'poolps{fh}') for fh in range(2)]
            blk_ps = {}
            gat_tiles = {}
            # per-group gathers; chunks reference their group's tile
            ch_by_grp = {}
            for ch in chunks:
                ch_by_grp.setdefault(ch['grp'], []).append(ch)

            pooled_n = [0]

            def finish_block(j, ps_pair, phase):
                if phase == 0:
                    # stage A partial (8x scale) into hA as fp8
                    for fh in range(2):
                        nc.scalar.activation(
                            hA[:, j * cfg.F + fh * 512:
                               j * cfg.F + (fh + 1) * 512],
                            ps_pair[fh][:], AF.Identity)
                else:
                    h_t = hp.tile([P, cfg.F], fp8, tag='h')
                    for fh in range(2):
                        nc.scalar.activation(
                            h_t[:, fh * 512:(fh + 1) * 512], ps_pair[fh][:],
                            AF.Lrelu, scale=0.125, alpha=0.01)
                    for fh in range(2):
                        nc.tensor.matmul(
                            pool_ps[fh][:],
                            lhsT=mpool_sb[:, j * cfg.B:(j + 1) * cfg.B],
                            rhs=h_t[:, fh * 512:(fh + 1) * 512],
                            start=(pooled_n[0] == 0),
                            stop=(pooled_n[0] == cfg.NBLK - 1))
                    pooled_n[0] += 1

            for g in range(n_groups):
                hf = groups[g][0]
                src = xwA if hf == 0 else xwB
                gat = gatp.tile([P, GRPU, cfg.F], fp8, tag='gat')
                nc.gpsimd.dma_gather(
                    out_ap=gat[:], in_ap=src[:, :],
                    idxs_ap=idx_sb[:, g * 64:(g + 1) * 64],
                    num_idxs=GRPU * P, num_idxs_reg=GRPU * P,
                    elem_size=cfg.F)
                for ch in ch_by_grp[g]:
                    j, u0 = ch['j'], ch['u0']
                    if ch['first']:
                        pair = [psB.tile([P, 512], f32, space="PSUM",
                                         name=f'blkps{fh}') for fh in range(2)]
                        blk_ps[(j, hf)] = pair
                        if hf == 1:
                            for fh in range(2):
                                nc.tensor.matmul(
                                    pair[fh][:], lhsT=id128[:],
                                    rhs=hA[:, j * cfg.F + fh * 512:
                                           j * cfg.F + (fh + 1) * 512],
                                    start=True, stop=False)
                    pair = blk_ps[(j, hf)]
                    sm0 = (g * GRPU + u0) * P
                    st = ch['first'] and hf == 0
                    sp = ch['last']
                    for fh in range(2):
                        if ch['units'] == 2 and USE_DR:
                            nc.tensor.matmul(
                                pair[fh][:],
                                lhsT=smat_sb[:, sm0:sm0 + 2 * P].rearrange(
                                    "p (i d) -> p i d", i=2),
                                rhs=gat[:, u0:u0 + 2,
                                        fh * 512:(fh + 1) * 512],
                                start=st, stop=sp, perf_mode=DR)
                        else:
                            for i in range(ch['units']):
                                nc.tensor.matmul(
                                    pair[fh][:],
                                    lhsT=smat_sb[:, sm0 + i * P:
                                                 sm0 + (i + 1) * P],
                                    rhs=gat[:, u0 + i,
                                            fh * 512:(fh + 1) * 512],
                                    start=(st and i == 0),
                                    stop=(sp and i == ch['units'] - 1))
                for ch in ch_by_grp[g]:
                    if ch['last']:
                        finish_block(ch['j'], blk_ps.pop((ch['j'], hf)), hf)

            # ---------------- pooled -> x1 partial
            pooled_sb = smp.tile([cfg.B, cfg.F], f32, tag='pooled')
            for fh in range(2):
                nc.scalar.activation(pooled_sb[:, fh * 512:(fh + 1) * 512],
                                     pool_ps[fh][:], AF.Identity,
                                     scale=float(2.0 ** -8))
            pfm = smp.tile([P, 8, cfg.B], f32, tag='pfm')
            for k in range(8):
                tps = psX.tile([P, cfg.B], f32, space="PSUM", tag='aux')
                nc.tensor.transpose(
                    out=tps[:], in_=pooled_sb[:, k * P:(k + 1) * P],
                    identity=id32[:])
                nc.scalar.activation(pfm[:, k, :], tps[:], AF.Identity)
            xps = psX.tile([P, cfg.B], f32, space="PSUM", tag='aux')
            for k in range(8):
                nc.tensor.matmul(xps[:], lhsT=wpf_sb[:, k, :],
                                 rhs=pfm[:, k, :],
                                 start=(k == 0), stop=(k == 7))
            x1p = smp.tile([P, cfg.B], f32, tag='x1p')
            nc.scalar.activation(x1p[:], xps[:], AF.Identity)

            # ---------------- cc packing + allreduce
            t_x1 = smp.tile([P, cfg.B], f32, tag='tx1')
            t_x2 = smp.tile([P, cfg.B], f32, tag='tx2')
            nc.scalar.activation(t_x1[:], x1p[:], AF.Identity,
                                 scale=mask1_sb[:, 0:1])
            nc.scalar.activation(t_x2[:], x1p[:], AF.Identity,
                                 scale=mask2_sb[:, 0:1])
            bounce_in = drp.tile([CC], f32, tag='ccin')
            bounce_out = drp.tile([N_CORES * CC], f32, tag='ccout')
            seg = P * cfg.B
            for i, t in enumerate((t_x1, t_x2, t_mas)):
                nc.sync.dma_start(
                    out=bounce_in[i * seg:(i + 1) * seg].rearrange(
                        "(p f) -> p f", f=cfg.B),
                    in_=t[:])
            nc.gpsimd.collective_compute(
                "AllGather", OP.bypass,
                replica_groups=[list(range(N_CORES))],
                ins=[bounce_in[:].opt()], outs=[bounce_out[:].opt()])
            gath_v = bounce_out[:].rearrange(
                "(r t p f) -> t p r f", r=N_CORES, t=3, p=P)

            def cc_sum(ti, name):
                raw = smp.tile([P, N_CORES, cfg.B], f32, tag='ccraw',
                               name=f'raw{name}')
                nc.sync.dma_start(out=raw[:], in_=gath_v[ti])
                red = smp.tile([P, cfg.B], f32, tag=f'ccred{name}',
                               name=f'red{name}')
                nc.vector.tensor_reduce(
                    out=red[:], in_=raw[:].transpose([0, 2, 1]),
                    axis=mybir.AxisListType.X, op=OP.add)
                return red

            x12 = {}
            for brr, bpf in ((1, bpf1_sb), (2, bpf2_sb)):
                xs = cc_sum(brr - 1, f'x{brr}')
                nc.scalar.activation(xs[:], xs[:], AF.Lrelu,
                                     bias=bpf[:, 0:1], alpha=0.01)
                x12[brr] = xs
            masif_rb = cc_sum(2, 'mas')

            # ---------------- head
            xc1 = {}
            for mh in range(2):
                cps = psX.tile([P, cfg.B], f32, space="PSUM", tag='aux')
                for k2 in range(2):
                    nc.tensor.matmul(
                        cps[:], lhsT=wfc1_sb[:, k2, mh * P:(mh + 1) * P],
                        rhs=x12[k2 + 1][:], start=(k2 == 0), stop=(k2 == 1))
                xcs = smp.tile([P, cfg.B], f32, tag=f'xc{mh}')
                nc.scalar.activation(xcs[:], cps[:], AF.Lrelu,
                                     bias=bfc1_sb[:, mh, 0:1], alpha=0.01)
                xc1[mh] = xcs
            c2ps = psX.tile([64, cfg.B], f32, space="PSUM", tag='aux')
            for k2 in range(2):
                nc.tensor.matmul(c2ps[:], lhsT=wfc2_sb[:, k2, :],
                                 rhs=xc1[k2][:], start=(k2 == 0),
                                 stop=(k2 == 1))
            xc = smp.tile([64, cfg.B], f32, tag='xcf')
            nc.scalar.activation(xc[:], c2ps[:], AF.Lrelu,
                                 bias=bfc2_sb[:, 0:1], alpha=0.01)

            ops = psX.tile([1, cfg.B], f32, space="PSUM", tag='aux')
            nc.tensor.matmul(ops[:], lhsT=wout_x_sb[:], rhs=xc[:],
                             start=True, stop=False)
            nc.tensor.matmul(ops[:], lhsT=wout_m_sb[:], rhs=masif_rb[:],
                             start=False, stop=True)
            res = smp.tile([1, cfg.B], f32, tag='res')
            nc.scalar.activation(res[:], ops[:], AF.Sigmoid,
                                 bias=bout_sb[:, 0:1])
            nc.sync.dma_start(out=out_t[:, :], in_=res[:])

    nc.compile()
    return nc


# ---------------------------------------------------------------- entry
_CACHE = {}


def _run(inputs, cfg, trace=False, tmpdir=None):
    from concourse import bass_utils
    meta, in_maps = _preprocess(inputs, cfg)
    key = tuple((c['hf'], c['j'], c['units'], c['u0'], c['grp'],
                 c['first'], c['last']) for c in meta['chunks'])
    if key not in _CACHE:
        _CACHE.clear()
        _CACHE[key] = _build(cfg, meta)
    nc = _CACHE[key]
    res = bass_utils.run_bass_kernel_spmd(
        nc, in_maps, core_ids=list(range(N_CORES)), trace=trace, tmpdir=tmpdir)
    out = np.asarray(res.results[0]['out'], np.float32).reshape(cfg.B, 1)
    return out, res


def kernel(**inputs) -> np.ndarray:
    cfg = _Cfg()
    out, _ = _run(inputs, cfg)
    return out
